# revision 1
# baseline (speedup 1.0000x reference)
"""Fused multi-head cross-attention with relation branch, sharded over 8 NeuronCores.

Sharding: data-parallel over batch (4) x tensor-parallel over head halves (2).
Core c handles batch c//2, heads [8*(c%2), 8*(c%2)+8). Each core computes its
partial output projection; the host sums the two partials per batch and adds bo.

Device data flow (per core):
  - q/k/rk projections emitted transposed: qT/kT/rkT [512 local dims, 1024 L]
    (4 chunks of 128 dims = head pairs (2dc, 2dc+1) at partitions 0-63/64-127)
  - v/rv projections emitted natural: [1024 LK, 512 dims], stored per lk-chunk
    with a ones column appended per head ([v_h | 1] of width 65) so the PV
    matmul's row 64 accumulates the softmax denominator for free.
  - scores computed transposed sT[lk, lq] = kT.T @ qT per head, two heads
    row-packed on the PE array (K=64 each at array rows 0-63 / 64-127).
  - exp + mask + 1/sqrt(dk) fused into one ACT op per score tile:
    p = exp(s*scale + bias[lk]) with bias = 0 / -1e9 from the key mask.
  - x_att^T accumulated in PSUM over lk chunks: [v_h|1].T @ p -> [65, lq].
  - softmax denominators batch-reciprocated on 128 DVE lanes via an SBUF->SBUF
    DMA reshape, broadcast over 64 partitions via a rank-1 PE matmul, then the
    two branches are combined with DVE fma ops.
  - output projection yT = WoT.T @ x_final accumulated over 4 dim chunks.

Matmul stage dtypes are configurable (bf16 for fast weight loads vs fp32r for
accuracy); PSUM accumulation is always fp32. The softmax-denominator
reciprocal/broadcast chain always stays fp32r.
"""

import math

import numpy as np

B, LQ, LK, D, H = 4, 1024, 1024, 1024, 16
DK = D // H
SCALE = 1.0 / math.sqrt(DK)
N_CORES = 8
HD = D // 2  # local dims per core (8 heads * 64)
# Keys are compacted host-side: only unmasked keys are shipped (padded to LKP
# with dummy rows whose mask bias is -1e9, so exp()=0 -> exact same math).
# mask ~ Bernoulli(1/2) over 1024 keys => valid ~ N(512, 16); 640 is +8 sigma.
LKP = 640
NM = LKP // 128  # lk chunks

_CACHE = {}

# Precision config: which matmul stages run bf16 (fast LDW) vs f32r (accurate).
CFG = {
    "score_bf16": True,  # query/key inputs, wq/wk, qT/kT/rkT score operands
    "v_bf16": True,      # value/rela inputs, wv/wrv/wrk, v_sb/rv_sb, p tiles
    "out_bf16": True,    # woT and x_final
}


def _build_program(lkp=LKP):
    import concourse.bacc as bacc
    import concourse.mybir as mybir
    import concourse.tile as tile

    LKP = lkp
    NM = LKP // 128

    f32 = mybir.dt.float32
    f32r = mybir.dt.float32r
    bf16 = mybir.dt.bfloat16
    Exp = mybir.ActivationFunctionType.Exp
    Add = mybir.AluOpType.add
    Mult = mybir.AluOpType.mult

    sdt = bf16 if CFG["score_bf16"] else f32
    vdt = bf16 if CFG["v_bf16"] else f32
    odt = bf16 if CFG["out_bf16"] else f32

    def mm_in(ap):
        """Matmul-operand view: f32 tiles feed the PE as f32r."""
        return ap.bitcast(f32r) if ap.dtype == f32 else ap

    def prod(ap):
        """Producer-output view for matmul-consumed tiles (verifier wants the
        producer to emit f32r when the consumer reads f32r)."""
        return ap.bitcast(f32r) if ap.dtype == f32 else ap

    nc = bacc.Bacc(
        "TRN2",
        target_bir_lowering=False,
        debug=False,
        enable_asserts=False,
        num_devices=N_CORES,
    )

    # DRAM I/O (per-core shapes; host shards/pre-transposes/casts).
    xqT = nc.dram_tensor("xqT", [D, LQ], sdt, kind="ExternalInput").ap()
    xkT = nc.dram_tensor("xkT", [D, LKP], sdt, kind="ExternalInput").ap()
    xrT = nc.dram_tensor("xrT", [D, LKP], vdt, kind="ExternalInput").ap()
    xvT = nc.dram_tensor("xvT", [D, LKP], vdt, kind="ExternalInput").ap()
    wqT = nc.dram_tensor("wqT", [D, HD], sdt, kind="ExternalInput").ap()
    wkT = nc.dram_tensor("wkT", [D, HD], sdt, kind="ExternalInput").ap()
    wrkT = nc.dram_tensor("wrkT", [D, HD], vdt, kind="ExternalInput").ap()
    wvT = nc.dram_tensor("wvT", [D, HD], vdt, kind="ExternalInput").ap()
    wrvT = nc.dram_tensor("wrvT", [D, HD], vdt, kind="ExternalInput").ap()
    woT = nc.dram_tensor("woT", [HD, D], odt, kind="ExternalInput").ap()
    bq_pc = nc.dram_tensor("bq_pc", [128, 4], f32, kind="ExternalInput").ap()
    bk_pc = nc.dram_tensor("bk_pc", [128, 4], f32, kind="ExternalInput").ap()
    brk_pc = nc.dram_tensor("brk_pc", [128, 4], f32, kind="ExternalInput").ap()
    bv_bc = nc.dram_tensor("bv_bc", [128, HD], f32, kind="ExternalInput").ap()
    brv_bc = nc.dram_tensor("brv_bc", [128, HD], f32, kind="ExternalInput").ap()
    maskb = nc.dram_tensor("maskb", [128, NM], f32, kind="ExternalInput").ap()
    ones_d = nc.dram_tensor("ones_d", [128, 64], f32, kind="ExternalInput").ap()
    yT = nc.dram_tensor("yT", [D, LQ], f32, kind="ExternalOutput").ap()
    scr1 = nc.dram_tensor("scr1", [8, 2048], f32, kind="Internal").ap()
    scr2 = nc.dram_tensor("scr2", [8, 2048], f32, kind="Internal").ap()

    def r(ap):
        return ap.bitcast(f32r)

    with tile.TileContext(nc) as tc:
        from contextlib import ExitStack

        with ExitStack() as ctx:
            # Persistent SBUF tensors.
            persist = ctx.enter_context(tc.tile_pool(name="persist", bufs=1))
            qT_sb = persist.tile([128, 4 * LQ], sdt, tag="qT")
            kT_sb = persist.tile([128, 4 * LKP], sdt, tag="kT")
            rkT_sb = persist.tile([128, 4 * LKP], sdt, tag="rkT")
            v_sb = persist.tile([128, NM * 8 * 65], vdt, tag="v")
            rv_sb = persist.tile([128, NM * 8 * 65], vdt, tag="rv")
            xf_sb = persist.tile([128, 4 * LQ], odt, tag="xf")
            ones_sb = persist.tile([128, 64], f32, tag="ones")
            maskb_sb = persist.tile([128, NM], f32, tag="maskb")
            bq_sb = persist.tile([128, 4], f32, tag="bq")
            bk_sb = persist.tile([128, 4], f32, tag="bk")
            brk_sb = persist.tile([128, 4], f32, tag="brk")
            bv_sb = persist.tile([128, HD], f32, tag="bv")
            brv_sb = persist.tile([128, HD], f32, tag="brv")

            nc.sync.dma_start(out=maskb_sb[:], in_=maskb)
            nc.sync.dma_start(out=bq_sb[:], in_=bq_pc)
            nc.sync.dma_start(out=bk_sb[:], in_=bk_pc)
            nc.sync.dma_start(out=brk_sb[:], in_=brk_pc)
            nc.sync.dma_start(out=bv_sb[:], in_=bv_bc)
            nc.sync.dma_start(out=brv_sb[:], in_=brv_bc)
            nc.sync.dma_start(out=ones_sb[:].bitcast(f32r), in_=ones_d.bitcast(f32r))

            v4 = v_sb[:].rearrange("p (m h c) -> p m h c", m=NM, h=8, c=65)
            rv4 = rv_sb[:].rearrange("p (m h c) -> p m h c", m=NM, h=8, c=65)
            if vdt == bf16:
                nc.vector.memset(v4[:, :, :, 64:65], 1.0)
                nc.vector.memset(rv4[:, :, :, 64:65], 1.0)
            else:
                oc = ones_sb[:, 0:NM * 8].rearrange("p (m h c) -> p m h c", m=NM, h=8, c=1)
                nc.vector.tensor_copy(out=prod(v4[:, :, :, 64:65]), in_=oc)
                nc.vector.tensor_copy(out=prod(rv4[:, :, :, 64:65]), in_=oc)

            # Score/exp pools opened BEFORE the projection pools so their PSUM
            # banks are disjoint from the projection psum banks (no false WAR:
            # scores may start as soon as q/k/rk chunks are ready).
            spool = ctx.enter_context(tc.tile_pool(name="spool", bufs=2, space="PSUM"))
            ppool = ctx.enter_context(tc.tile_pool(name="ppool", bufs=18))

            p_tiles = {}

            def emit_scores(lqh):
                for dc in range(4):
                    qsl = slice(1024 * dc + 512 * lqh, 1024 * dc + 512 * lqh + 512)
                    for m in range(NM):
                        ksl = slice(LKP * dc + 128 * m, LKP * dc + 128 * m + 128)
                        for br, kt in ((0, kT_sb), (1, rkT_sb)):
                            s = spool.tile([128, 1024], f32, tag="spool", name="s")
                            nc.tensor.matmul(
                                s[:, 0:512], mm_in(kt[0:64, ksl]), mm_in(qT_sb[0:64, qsl])
                            )
                            nc.tensor.matmul(
                                s[:, 512:1024],
                                mm_in(kt[64:128, ksl]),
                                mm_in(qT_sb[64:128, qsl]),
                            )
                            p = ppool.tile([128, 1024], vdt, tag="ppool", name="p")
                            nc.scalar.activation(
                                prod(p[:]),
                                s[:],
                                Exp,
                                bias=maskb_sb[:, m : m + 1],
                                scale=SCALE,
                            )
                            p_tiles[(lqh, dc, m, br)] = p


            # ---------------- Phase 1: projections ----------------
            with ExitStack() as ph1:
                inp = ph1.enter_context(tc.tile_pool(name="inp", bufs=16))
                wch_pool = ph1.enter_context(tc.tile_pool(name="wch", bufs=12))
                ppsum = ph1.enter_context(
                    tc.tile_pool(name="ppsum", bufs=2, space="PSUM")
                )

                # Transposed projections: out chunk dc = lhsT(W block).T @ x_chunk.
                for name, xt, wt, b_sb, out_sb, dt_, LL in (
                    ("q", xqT, wqT, bq_sb, qT_sb, sdt, LQ),
                    ("k", xkT, wkT, bk_sb, kT_sb, sdt, LKP),
                    ("rk", xrT, wrkT, brk_sb, rkT_sb, vdt, LKP),
                ):
                    nsl = [slice(a, min(a + 512, LL)) for a in range(0, LL, 512)]
                    xch = []
                    wch = []
                    for k in range(8):
                        t = inp.tile([128, LL], dt_, tag="inp", name=f"x{name}{k}")
                        nc.sync.dma_start(
                            out=prod(t[:]), in_=mm_in(xt[128 * k : 128 * k + 128, :])
                        )
                        xch.append(t)
                        w = wch_pool.tile([128, HD], dt_, tag="wch", name=f"w{name}{k}")
                        nc.sync.dma_start(
                            out=prod(w[:]), in_=mm_in(wt[128 * k : 128 * k + 128, :])
                        )
                        wch.append(w)
                    for dc in range(4):
                        ps = ppsum.tile([128, LL], f32, tag="ppsum")
                        for k in range(8):
                            for sl in nsl:
                                nc.tensor.matmul(
                                    ps[:, sl],
                                    mm_in(wch[k][:, 128 * dc : 128 * dc + 128]),
                                    mm_in(xch[k][:, sl]),
                                    start=(k == 0),
                                    stop=(k == 7),
                                )
                        nc.vector.tensor_scalar(
                            out=prod(out_sb[:, LL * dc : LL * dc + LL]),
                            in0=ps[:],
                            scalar1=b_sb[:, dc : dc + 1],
                            scalar2=None,
                            op0=Add,
                        )

                # Scores/exp for the first lq half can start as soon as the
                # q/k/rk projections land - emit them before v/rv so the ACT
                # engine gets fed during the remaining projections.
                emit_scores(0)

                # Natural-orientation projections for v / rv.
                for name, xt, wt, b_sb, out4 in (
                    ("v", xvT, wvT, bv_sb, v4),
                    ("rv", xrT, wrvT, brv_sb, rv4),
                ):
                    xch = []
                    wch = []
                    for k in range(8):
                        t = inp.tile([128, LKP], vdt, tag="inp", name=f"x{name}{k}")
                        nc.sync.dma_start(
                            out=prod(t[:]), in_=mm_in(xt[128 * k : 128 * k + 128, :])
                        )
                        xch.append(t)
                        w = wch_pool.tile([128, HD], vdt, tag="wch", name=f"w{name}{k}")
                        nc.sync.dma_start(
                            out=prod(w[:]), in_=mm_in(wt[128 * k : 128 * k + 128, :])
                        )
                        wch.append(w)
                    for m in range(NM):
                        ps = ppsum.tile([128, 512], f32, tag="ppsum")
                        for k in range(8):
                            nc.tensor.matmul(
                                ps[:],
                                mm_in(xch[k][:, 128 * m : 128 * m + 128]),
                                mm_in(wch[k][:]),
                                start=(k == 0),
                                stop=(k == 7),
                            )
                        nc.vector.tensor_tensor(
                            out=prod(out4[:, m, :, 0:64]),
                            in0=ps[:].rearrange("p (h c) -> p h c", h=8, c=64),
                            in1=b_sb[:].rearrange("p (h c) -> p h c", h=8, c=64),
                            op=Add,
                        )

            # -------- Phase A: projections emitted above; now scores+exp --------
            # (spool opened below coexists with ppsum inside ph1? No: ph1 already
            # closed at this indentation. Scores/exp are emitted here, before the
            # PV pass, so the ACT engine can start as soon as q/k/rk chunks land.)

            emit_scores(1)

            # -------- Phase B: PV accumulation, normalize, output projection ----
            with ExitStack() as ph2:
                xpool = ph2.enter_context(
                    tc.tile_pool(name="xpool", bufs=4, space="PSUM")
                )
                xsb = ph2.enter_context(tc.tile_pool(name="xsb", bufs=8))
                sgp = ph2.enter_context(tc.tile_pool(name="sgp", bufs=2))
                bcp = ph2.enter_context(tc.tile_pool(name="bcp", bufs=4))
                wop = ph2.enter_context(tc.tile_pool(name="wop", bufs=4))
                ysb = ph2.enter_context(tc.tile_pool(name="ysb", bufs=4))

                woch = []
                for dc in range(4):
                    w = wop.tile([128, 1024], odt, tag="wop", name=f"wo{dc}")
                    nc.sync.dma_start(
                        out=prod(w[:]), in_=mm_in(woT[128 * dc : 128 * dc + 128, :])
                    )
                    woch.append(w)

                def emit_outproj(lqh, wide=False):
                    if not wide:
                        for ot in range(8):
                            ps = xpool.tile(
                                [128, 512], f32, tag="xpool", name=f"psy{ot}"
                            )
                            for dc in range(4):
                                nc.tensor.matmul(
                                    ps[:],
                                    mm_in(woch[dc][:, 128 * ot : 128 * ot + 128]),
                                    mm_in(
                                        xf_sb[
                                            :,
                                            1024 * dc
                                            + 512 * lqh : 1024 * dc
                                            + 512 * lqh
                                            + 512,
                                        ]
                                    ),
                                    start=(dc == 0),
                                    stop=(dc == 3),
                                )
                            y = ysb.tile([128, 512], f32, tag="ysb")
                            nc.vector.tensor_copy(out=y[:], in_=ps[:])
                            nc.sync.dma_start(
                                out=yT[
                                    128 * ot : 128 * ot + 128,
                                    512 * lqh : 512 * lqh + 512,
                                ],
                                in_=y[:],
                            )
                        return
                    # Wide variant: all 8 ot accumulators live at once (4 xpool
                    # banks + 2 idle spool slots split in half), dc-outer so only
                    # the final dim-chunk's 8 matmuls wait on the last normalize.
                    pss = []
                    for i in range(4):
                        pss.append(
                            xpool.tile([128, 512], f32, tag="xpool", name=f"psw{i}")
                        )
                    for i in range(2):
                        w2 = spool.tile([128, 1024], f32, tag="spool", name=f"psw2{i}")
                        pss.append(w2[:, 0:512])
                        pss.append(w2[:, 512:1024])
                    for dc in range(4):
                        for ot in range(8):
                            nc.tensor.matmul(
                                pss[ot],
                                mm_in(woch[dc][:, 128 * ot : 128 * ot + 128]),
                                mm_in(
                                    xf_sb[
                                        :,
                                        1024 * dc
                                        + 512 * lqh : 1024 * dc
                                        + 512 * lqh
                                        + 512,
                                    ]
                                ),
                                start=(dc == 0),
                                stop=(dc == 3),
                            )
                    for ot in range(8):
                        y = ysb.tile([128, 512], f32, tag="ysb")
                        nc.vector.tensor_copy(out=y[:], in_=pss[ot])
                        nc.sync.dma_start(
                            out=yT[
                                128 * ot : 128 * ot + 128, 512 * lqh : 512 * lqh + 512
                            ],
                            in_=y[:],
                        )

                for lqh in range(2):
                    for dc in range(4):
                        if lqh == 1 and dc == 2:
                            emit_outproj(0)
                        xacc = {}
                        for br in range(2):
                            for hs in range(2):
                                xacc[(br, hs)] = xpool.tile(
                                    [65, 512], f32, tag="xpool", name=f"xacc{br}{hs}"
                                )
                        for m in range(NM):
                            for br, vv in ((0, v4), (1, rv4)):
                                pt = p_tiles[(lqh, dc, m, br)]
                                for hs in range(2):
                                    nc.tensor.matmul(
                                        xacc[(br, hs)][:],
                                        mm_in(vv[:, m, 2 * dc + hs, :]),
                                        mm_in(pt[:, 512 * hs : 512 * hs + 512]),
                                        start=(m == 0),
                                        stop=(m == NM - 1),
                                    )
                        # Copy x accumulators (with sums in row 64) to SBUF,
                        # packed into one tile so the denominator row ships to
                        # DRAM in a single DMA.
                        xs_all = xsb.tile([65, 4 * 512], f32, tag="xsall", bufs=3)
                        xs = {}
                        for j, (br, hs) in enumerate(
                            [(0, 0), (1, 0), (0, 1), (1, 1)]
                        ):
                            sl = xs_all[:, 512 * j : 512 * j + 512]
                            nc.vector.tensor_copy(out=sl, in_=xacc[(br, hs)][:])
                            xs[(br, hs)] = sl
                        # Batch-reciprocate the 4 denominator rows via DRAM.
                        it = 2 * dc + lqh
                        sg = sgp.tile([128, 16], f32, tag="sgp")
                        nc.sync.dma_start(out=scr1[it, :], in_=xs_all[64:65, :])
                        nc.sync.dma_start(out=sg[:], in_=scr1[it, :])
                        nc.vector.reciprocal(sg[:], sg[:])
                        nc.sync.dma_start(out=scr2[it, :], in_=sg[:])
                        for hs in range(2):
                            jv, jr = 2 * hs, 2 * hs + 1
                            bcv = bcp.tile([64, 512], f32, tag="bcp", name="bcv")
                            nc.gpsimd.dma_start(
                                out=bcv[:],
                                in_=scr2[it : it + 1, 512 * jv : 512 * jv + 512]
                                .partition_broadcast(64)[:, 0, :],
                            )
                            bcr = bcp.tile([64, 512], f32, tag="bcp", name="bcr")
                            nc.gpsimd.dma_start(
                                out=bcr[:],
                                in_=scr2[it : it + 1, 512 * jr : 512 * jr + 512]
                                .partition_broadcast(64)[:, 0, :],
                            )
                            t1 = xsb.tile([65, 512], f32, tag="xsb")
                            nc.vector.tensor_tensor(
                                out=t1[0:64, :],
                                in0=xs[(0, hs)][0:64, :],
                                in1=bcv[:],
                                op=Mult,
                            )
                            t2 = xsb.tile([65, 512], f32, tag="xsb")
                            nc.vector.tensor_tensor(
                                out=t2[0:64, :],
                                in0=xs[(1, hs)][0:64, :],
                                in1=bcr[:],
                                op=Mult,
                            )
                            xf_slice = slice(
                                1024 * dc + 512 * lqh, 1024 * dc + 512 * lqh + 512
                            )
                            if hs == 0:
                                nc.vector.tensor_tensor(
                                    out=prod(xf_sb[0:64, xf_slice]),
                                    in0=t1[0:64, :],
                                    in1=t2[0:64, :],
                                    op=Add,
                                )
                            else:
                                t3 = xsb.tile([65, 512], odt, tag="xsb")
                                nc.vector.tensor_tensor(
                                    out=prod(t3[0:64, :]),
                                    in0=t1[0:64, :],
                                    in1=t2[0:64, :],
                                    op=Add,
                                )
                                nc.sync.dma_start(
                                    out=prod(xf_sb[64:128, xf_slice]),
                                    in_=prod(t3[0:64, :]),
                                )
                emit_outproj(1, wide=True)

    nc.compile()
    return nc


def _get_program(lkp=LKP):
    if lkp not in _CACHE:
        _CACHE[lkp] = _build_program(lkp)
    return _CACHE[lkp]


def _cast(arr, bf16_flag):
    if bf16_flag:
        import ml_dtypes

        return np.ascontiguousarray(arr.astype(ml_dtypes.bfloat16))
    return np.ascontiguousarray(arr.astype(np.float32))


def _shard_inputs(inputs, lkp=LKP):
    q = np.ascontiguousarray(inputs["query"], dtype=np.float32)
    k = np.ascontiguousarray(inputs["key"], dtype=np.float32)
    v = np.ascontiguousarray(inputs["value"], dtype=np.float32)
    wr = np.ascontiguousarray(inputs["weak_rela"], dtype=np.float32)
    mask = np.asarray(inputs["mask"])
    sb, vb, ob = CFG["score_bf16"], CFG["v_bf16"], CFG["out_bf16"]

    in_maps = []
    for c in range(N_CORES):
        b, hh = divmod(c, 2)
        hsl = slice(HD * hh, HD * hh + HD)
        idx = np.nonzero(mask[b, 0])[0]
        nv = len(idx)
        assert nv <= lkp
        pidx = np.concatenate([idx, np.zeros(lkp - nv, dtype=idx.dtype)])
        bias = np.full(lkp, -1.0e9, np.float32)
        bias[:nv] = 0.0
        mb = np.ascontiguousarray(bias.reshape(lkp // 128, 128).T)
        kc, vc, wrc = k[b][pidx], v[b][pidx], wr[b][pidx]
        m = {
            "xqT": _cast(q[b].T, sb),
            "xkT": _cast(kc.T, sb),
            "xrT": _cast(wrc.T, vb),
            "xvT": _cast(vc.T, vb),
            "wqT": _cast(np.asarray(inputs["Wq"])[hsl, :].T, sb),
            "wkT": _cast(np.asarray(inputs["Wk"])[hsl, :].T, sb),
            "wrkT": _cast(np.asarray(inputs["Wrk"])[hsl, :].T, vb),
            "wvT": _cast(np.asarray(inputs["Wv"])[hsl, :].T, vb),
            "wrvT": _cast(np.asarray(inputs["Wrv"])[hsl, :].T, vb),
            "woT": _cast(np.asarray(inputs["Wo"])[:, hsl].T, ob),
            "bq_pc": np.asarray(inputs["bq"][hsl])
            .reshape(4, 128)
            .T.astype(np.float32),
            "bk_pc": np.asarray(inputs["bk"][hsl])
            .reshape(4, 128)
            .T.astype(np.float32),
            "brk_pc": np.asarray(inputs["brk"][hsl])
            .reshape(4, 128)
            .T.astype(np.float32),
            "bv_bc": np.broadcast_to(inputs["bv"][hsl], (128, HD)).astype(np.float32),
            "brv_bc": np.broadcast_to(inputs["brv"][hsl], (128, HD)).astype(
                np.float32
            ),
            "maskb": mb,
            "ones_d": np.ones((128, 64), np.float32),
        }
        in_maps.append({k2: np.ascontiguousarray(v2) for k2, v2 in m.items()})
    return in_maps


def run_on_hw(inputs, trace=False, **kw):
    from concourse.bass_utils import run_bass_kernel_spmd

    mask = np.asarray(inputs["mask"])
    max_valid = max(int(mask[b, 0].sum()) for b in range(B))
    lkp = max(LKP, ((max_valid + 127) // 128) * 128)
    nc = _get_program(lkp)
    in_maps = _shard_inputs(inputs, lkp)
    res = run_bass_kernel_spmd(
        nc, in_maps, core_ids=list(range(N_CORES)), trace=trace, **kw
    )
    bo = np.asarray(inputs["bo"], dtype=np.float32)
    outs = []
    for b in range(B):
        yt = res.results[2 * b]["yT"] + res.results[2 * b + 1]["yT"]
        outs.append(yt.T + bo)
    out = np.stack(outs).astype(np.float32)
    return out, res


def kernel(**inputs):
    out, _ = run_on_hw(inputs)
    return out



# revision 7
# speedup vs baseline: 1.0130x; 1.0130x over previous
"""Fused multi-head cross-attention with relation branch, sharded over 8 NeuronCores.

Sharding: data-parallel over batch (4) x tensor-parallel over head halves (2).
Core c handles batch c//2, heads [8*(c%2), 8*(c%2)+8). Each core computes its
partial output projection; the host sums the two partials per batch and adds bo.

v2 structure (vs v1):
  - Host pre-packs every input into its device SBUF layout ([128, N] tiles),
    so each tensor loads with ONE large DMA descriptor (packets round-robin
    over all 16 DMA engines, so big descriptors lose no bandwidth). Issue is
    split across the two HWDGE queues (sync + scalar) plus gpsimd for the
    small persistent tensors, with critical-path tensors first.
  - dc-major phase 1: for each dim-chunk dc, project q/k/rk then immediately
    emit that chunk's scores+exp, so the ACT engine (co-critical at ~74us)
    starts ~15us earlier than a tensor-major ordering.
  - Scores for one (lqh, dc, m) are 4 matmuls (2 branches x 2 row-tiled K=64
    head matmuls) into one [128, 2048] PSUM tile, exp'd by a single ACTIVATE
    (the mask bias is shared by both branches).
  - Softmax normalize chain runs in bf16 (2x DVE rate): PSUM->SBUF copies,
    DRAM-roundtrip batched reciprocal, gpsimd partition-broadcasts, fma.
  - The last PV iteration's accumulators live in spool (free after the last
    exp) so the output projection for lqh=0 - emitted AFTER all PV work -
    can run on xpool banks while the final normalize chain drains.
  - yT is written bf16 (host upcasts, sums partials, adds bo in f32).
"""

import math

import numpy as np

B, LQ, LK, D, H = 4, 1024, 1024, 1024, 16
DK = D // H
SCALE = 1.0 / math.sqrt(DK)
N_CORES = 8
HD = D // 2  # local dims per core (8 heads * 64)
# Keys are compacted host-side: only unmasked keys are shipped (padded to LKP
# with dummy rows whose mask bias is -1e9, so exp()=0 -> exact same math).
LKP = 640
NM = LKP // 128  # lk chunks

_CACHE = {}


def _build_program(lkp=LKP):
    import concourse.bacc as bacc
    import concourse.mybir as mybir
    import concourse.tile as tile

    LKP = lkp
    NM = LKP // 128

    f32 = mybir.dt.float32
    bf16 = mybir.dt.bfloat16
    Exp = mybir.ActivationFunctionType.Exp
    Add = mybir.AluOpType.add
    Mult = mybir.AluOpType.mult

    nc = bacc.Bacc(
        "TRN2",
        target_bir_lowering=False,
        debug=False,
        enable_asserts=False,
        num_devices=N_CORES,
    )

    # DRAM I/O. Everything arrives pre-packed in its SBUF layout.
    # x tensors: [128, 8*L] with k-chunk k at cols [k*L, (k+1)*L).
    xq_d = nc.dram_tensor("xq", [128, 8 * LQ], bf16, kind="ExternalInput").ap()
    xk_d = nc.dram_tensor("xk", [128, 8 * LKP], bf16, kind="ExternalInput").ap()
    xr_d = nc.dram_tensor("xr", [128, 8 * LKP], bf16, kind="ExternalInput").ap()
    xv_d = nc.dram_tensor("xv", [128, 8 * LKP], bf16, kind="ExternalInput").ap()
    # transposed-proj weights: [128, 4*8*128]: col = dc*1024 + k*128 + c
    wq_d = nc.dram_tensor("wq", [128, 4096], bf16, kind="ExternalInput").ap()
    wk_d = nc.dram_tensor("wk", [128, 4096], bf16, kind="ExternalInput").ap()
    wrk_d = nc.dram_tensor("wrk", [128, 4096], bf16, kind="ExternalInput").ap()
    # natural-proj weights: [128, 8*512]: col = k*512 + c
    wv_d = nc.dram_tensor("wv", [128, 4096], bf16, kind="ExternalInput").ap()
    wrv_d = nc.dram_tensor("wrv", [128, 4096], bf16, kind="ExternalInput").ap()
    # output-proj weights: [128, 4*1024]: col = dc*1024 + c
    wo_d = nc.dram_tensor("wo", [128, 4096], bf16, kind="ExternalInput").ap()
    bq_pc = nc.dram_tensor("bq_pc", [128, 4], f32, kind="ExternalInput").ap()
    bk_pc = nc.dram_tensor("bk_pc", [128, 4], f32, kind="ExternalInput").ap()
    brk_pc = nc.dram_tensor("brk_pc", [128, 4], f32, kind="ExternalInput").ap()
    bv_bc = nc.dram_tensor("bv_bc", [128, HD], f32, kind="ExternalInput").ap()
    brv_bc = nc.dram_tensor("brv_bc", [128, HD], f32, kind="ExternalInput").ap()
    maskb = nc.dram_tensor("maskb", [128, NM], f32, kind="ExternalInput").ap()
    yT = nc.dram_tensor("yT", [D, LQ], bf16, kind="ExternalOutput").ap()
    scr1 = nc.dram_tensor("scr1", [8, 2048], bf16, kind="Internal").ap()
    scr2 = nc.dram_tensor("scr2", [8, 2048], bf16, kind="Internal").ap()

    with nc.allow_low_precision(
        reason="bf16 normalize pipeline by design; matmul PSUM acc stays f32"
    ), tile.TileContext(nc) as tc:
        from contextlib import ExitStack

        with ExitStack() as ctx:
            # ---- persistent SBUF tensors (whole-program lifetime) ----
            persist = ctx.enter_context(tc.tile_pool(name="persist", bufs=1))
            qT_sb = persist.tile([128, 4 * LQ], bf16, tag="qT")
            kT_sb = persist.tile([128, 4 * LKP], bf16, tag="kT")
            rkT_sb = persist.tile([128, 4 * LKP], bf16, tag="rkT")
            v_sb = persist.tile([128, NM * 8 * 65], bf16, tag="v")
            rv_sb = persist.tile([128, NM * 8 * 65], bf16, tag="rv")
            xf_sb = persist.tile([128, 4 * LQ], bf16, tag="xf")
            wo_sb = persist.tile([128, 4096], bf16, tag="wo")
            maskb_sb = persist.tile([128, NM], f32, tag="maskb")
            bq_sb = persist.tile([128, 4], f32, tag="bq")
            bk_sb = persist.tile([128, 4], f32, tag="bk")
            brk_sb = persist.tile([128, 4], f32, tag="brk")
            bv_sb = persist.tile([128, HD], f32, tag="bv")
            brv_sb = persist.tile([128, HD], f32, tag="brv")

            v4 = v_sb[:].rearrange("p (m h c) -> p m h c", m=NM, h=8, c=65)
            rv4 = rv_sb[:].rearrange("p (m h c) -> p m h c", m=NM, h=8, c=65)
            nc.vector.memset(v4[:, :, :, 64:65], 1.0)
            nc.vector.memset(rv4[:, :, :, 64:65], 1.0)

            # score PSUM (4 banks) + p-tile pool; 16 quad bufs of lookahead
            spool = ctx.enter_context(tc.tile_pool(name="spool", bufs=1, space="PSUM"))
            ppool = ctx.enter_context(tc.tile_pool(name="ppool", bufs=16))
            p_tiles = {}

            def emit_score_quad(lqh, dc, m):
                qsl = slice(1024 * dc + 512 * lqh, 1024 * dc + 512 * lqh + 512)
                ksl = slice(LKP * dc + 128 * m, LKP * dc + 128 * m + 128)
                s = spool.tile([128, 2048], f32, tag="spool", name="s")
                nc.tensor.matmul(s[:, 0:512], kT_sb[0:64, ksl], qT_sb[0:64, qsl])
                nc.tensor.matmul(
                    s[:, 512:1024], kT_sb[64:128, ksl], qT_sb[64:128, qsl]
                )
                nc.tensor.matmul(
                    s[:, 1024:1536], rkT_sb[0:64, ksl], qT_sb[0:64, qsl]
                )
                nc.tensor.matmul(
                    s[:, 1536:2048], rkT_sb[64:128, ksl], qT_sb[64:128, qsl]
                )
                p = ppool.tile([128, 2048], bf16, tag="ppool", name="p")
                nc.scalar.activation(
                    p[:], s[:], Exp, bias=maskb_sb[:, m : m + 1], scale=SCALE
                )
                p_tiles[(lqh, dc, m)] = p

            # ---------------- Phase 1: dc-major q/k/rk + scores(0) ----------
            with ExitStack() as ph1:
                ppsum = ph1.enter_context(
                    tc.tile_pool(name="ppsum", bufs=2, space="PSUM")
                )
                # input tensors live only through phase 1; phase-2 pools
                # reuse their SBUF space
                xin = ph1.enter_context(tc.tile_pool(name="xin", bufs=1))
                xq_sb = xin.tile([128, 8 * LQ], bf16, tag="xq")
                xk_sb = xin.tile([128, 8 * LKP], bf16, tag="xk")
                xr_sb = xin.tile([128, 8 * LKP], bf16, tag="xr")
                xv_sb = xin.tile([128, 8 * LKP], bf16, tag="xv")
                wq_sb = xin.tile([128, 4096], bf16, tag="wq")
                wk_sb = xin.tile([128, 4096], bf16, tag="wk")
                wrk_sb = xin.tile([128, 4096], bf16, tag="wrk")
                wv_sb = xin.tile([128, 4096], bf16, tag="wv")
                wrv_sb = xin.tile([128, 4096], bf16, tag="wrv")

                # ---- input DMAs: sync queue carries the critical path ----
                # (descriptor issue costs ~0.6us each on the issuing queue,
                # but one descriptor's packets spread over all 16 DMA engines)
                nc.sync.dma_start(out=xq_sb[:], in_=xq_d)
                nc.sync.dma_start(out=wq_sb[:, 0:1024], in_=wq_d[:, 0:1024])
                nc.sync.dma_start(out=xk_sb[:], in_=xk_d)
                nc.sync.dma_start(out=wk_sb[:, 0:1024], in_=wk_d[:, 0:1024])
                nc.sync.dma_start(out=xr_sb[:], in_=xr_d)
                nc.sync.dma_start(out=wrk_sb[:, 0:1024], in_=wrk_d[:, 0:1024])
                nc.sync.dma_start(out=wq_sb[:, 1024:4096], in_=wq_d[:, 1024:4096])
                nc.sync.dma_start(out=wk_sb[:, 1024:4096], in_=wk_d[:, 1024:4096])
                nc.sync.dma_start(out=wrk_sb[:, 1024:4096], in_=wrk_d[:, 1024:4096])
                # scalar (2nd HWDGE) queue: v-branch + out weights + biases
                nc.scalar.dma_start(out=xv_sb[:], in_=xv_d)
                nc.scalar.dma_start(out=wv_sb[:], in_=wv_d)
                nc.scalar.dma_start(out=wrv_sb[:], in_=wrv_d)
                nc.scalar.dma_start(out=wo_sb[:], in_=wo_d)
                nc.scalar.dma_start(out=bv_sb[:], in_=bv_bc)
                nc.scalar.dma_start(out=brv_sb[:], in_=brv_bc)
                # gpsimd (software DGE): tiny persistent tensors
                nc.gpsimd.dma_start(out=maskb_sb[:], in_=maskb)
                nc.gpsimd.dma_start(out=bq_sb[:], in_=bq_pc)
                nc.gpsimd.dma_start(out=bk_sb[:], in_=bk_pc)
                nc.gpsimd.dma_start(out=brk_sb[:], in_=brk_pc)

                def tproj(dc, x_sb, w_sb, b_sb, out_sb, LL):
                    nsl = [slice(a, min(a + 512, LL)) for a in range(0, LL, 512)]
                    ps = ppsum.tile([128, LL], f32, tag="ppsum")
                    for k in range(8):
                        wsl = w_sb[:, 1024 * dc + 128 * k : 1024 * dc + 128 * k + 128]
                        for sl in nsl:
                            nc.tensor.matmul(
                                ps[:, sl],
                                wsl,
                                x_sb[:, LL * k + sl.start : LL * k + sl.stop],
                                start=(k == 0),
                                stop=(k == 7),
                            )
                    nc.vector.tensor_scalar(
                        out=out_sb[:, LL * dc : LL * dc + LL],
                        in0=ps[:],
                        scalar1=b_sb[:, dc : dc + 1],
                        scalar2=None,
                        op0=Add,
                    )

                for dc in range(4):
                    tproj(dc, xq_sb, wq_sb, bq_sb, qT_sb, LQ)
                    tproj(dc, xk_sb, wk_sb, bk_sb, kT_sb, LKP)
                    tproj(dc, xr_sb, wrk_sb, brk_sb, rkT_sb, LKP)
                    for m in range(NM):
                        emit_score_quad(0, dc, m)

                # scores for the second lq half (exp keeps the ACT engine fed
                # while the v/rv projections run on PE)
                for dc in range(4):
                    for m in range(NM):
                        emit_score_quad(1, dc, m)

                # Natural-orientation projections for v / rv.
                for x_sb, w_sb, b_sb, out4 in (
                    (xv_sb, wv_sb, bv_sb, v4),
                    (xr_sb, wrv_sb, brv_sb, rv4),
                ):
                    for m in range(NM):
                        ps = ppsum.tile([128, 512], f32, tag="ppsum")
                        for k in range(8):
                            nc.tensor.matmul(
                                ps[:],
                                x_sb[:, LKP * k + 128 * m : LKP * k + 128 * m + 128],
                                w_sb[:, 512 * k : 512 * k + 512],
                                start=(k == 0),
                                stop=(k == 7),
                            )
                        nc.vector.tensor_tensor(
                            out=out4[:, m, :, 0:64],
                            in0=ps[:].rearrange("p (h c) -> p h c", h=8, c=64),
                            in1=b_sb[:].rearrange("p (h c) -> p h c", h=8, c=64),
                            op=Add,
                        )

            # -------- Phase 2: PV accumulation, normalize, output projection --
            with ExitStack() as ph2:
                xpool = ph2.enter_context(
                    tc.tile_pool(name="xpool", bufs=4, space="PSUM")
                )
                xsb = ph2.enter_context(tc.tile_pool(name="xsb", bufs=6))
                sgp = ph2.enter_context(tc.tile_pool(name="sgp", bufs=2))
                bcp = ph2.enter_context(tc.tile_pool(name="bcp", bufs=4))
                ysb = ph2.enter_context(tc.tile_pool(name="ysb", bufs=4))

                def pv_iter(lqh, dc, use_spool):
                    it = 2 * dc + lqh
                    xacc = {}
                    if use_spool:
                        w2 = spool.tile([128, 2048], f32, tag="spool", name="xaccs")
                        for j, (br, hs) in enumerate(
                            [(0, 0), (0, 1), (1, 0), (1, 1)]
                        ):
                            xacc[(br, hs)] = w2[:, 512 * j : 512 * j + 512]
                    else:
                        for br in range(2):
                            for hs in range(2):
                                xacc[(br, hs)] = xpool.tile(
                                    [65, 512], f32, tag="xpool", name=f"xacc{br}{hs}"
                                )[:, :]
                    for m in range(NM):
                        pt = p_tiles[(lqh, dc, m)]
                        for br, vv in ((0, v4), (1, rv4)):
                            for hs in range(2):
                                nc.tensor.matmul(
                                    xacc[(br, hs)][0:65, :],
                                    vv[:, m, 2 * dc + hs, :],
                                    pt[:, 1024 * br + 512 * hs : 1024 * br + 512 * hs + 512],
                                    start=(m == 0),
                                    stop=(m == NM - 1),
                                )
                    # Evict accumulators (bf16) so the PSUM banks free fast;
                    # row 64 carries the softmax denominators.
                    xs_all = xsb.tile([65, 2048], bf16, tag="xsall", bufs=3)
                    xs = {}
                    for j, (br, hs) in enumerate([(0, 0), (1, 0), (0, 1), (1, 1)]):
                        sl = xs_all[:, 512 * j : 512 * j + 512]
                        nc.vector.tensor_copy(out=sl, in_=xacc[(br, hs)][0:65, :])
                        xs[(br, hs)] = sl
                    # Batch-reciprocate the 4 denominator rows via DRAM (the
                    # [1,2048] row respreads over 128 DVE lanes as [128,16]).
                    sg = sgp.tile([128, 16], bf16, tag="sgp")
                    nc.sync.dma_start(out=scr1[it, :], in_=xs_all[64:65, :])
                    nc.sync.dma_start(out=sg[:], in_=scr1[it, :])
                    nc.vector.reciprocal(sg[:], sg[:])
                    nc.sync.dma_start(out=scr2[it, :], in_=sg[:])
                    for hs in range(2):
                        jv, jr = 2 * hs, 2 * hs + 1
                        bcv = bcp.tile([64, 512], bf16, tag="bcp", name="bcv")
                        nc.gpsimd.dma_start(
                            out=bcv[:],
                            in_=scr2[it : it + 1, 512 * jv : 512 * jv + 512]
                            .partition_broadcast(64)[:, 0, :],
                        )
                        bcr = bcp.tile([64, 512], bf16, tag="bcp", name="bcr")
                        nc.gpsimd.dma_start(
                            out=bcr[:],
                            in_=scr2[it : it + 1, 512 * jr : 512 * jr + 512]
                            .partition_broadcast(64)[:, 0, :],
                        )
                        t1 = xsb.tile([65, 512], bf16, tag="xsb")
                        nc.vector.tensor_tensor(
                            out=t1[0:64, :], in0=xs[(0, hs)][0:64, :], in1=bcv[:],
                            op=Mult,
                        )
                        t2 = xsb.tile([65, 512], bf16, tag="xsb")
                        nc.vector.tensor_tensor(
                            out=t2[0:64, :], in0=xs[(1, hs)][0:64, :], in1=bcr[:],
                            op=Mult,
                        )
                        xf_slice = slice(
                            1024 * dc + 512 * lqh, 1024 * dc + 512 * lqh + 512
                        )
                        if hs == 0:
                            nc.vector.tensor_tensor(
                                out=xf_sb[0:64, xf_slice], in0=t1[0:64, :],
                                in1=t2[0:64, :], op=Add,
                            )
                        else:
                            t3 = xsb.tile([65, 512], bf16, tag="xsb")
                            nc.vector.tensor_tensor(
                                out=t3[0:64, :], in0=t1[0:64, :], in1=t2[0:64, :],
                                op=Add,
                            )
                            nc.sync.dma_start(
                                out=xf_sb[64:128, xf_slice], in_=t3[0:64, :]
                            )

                def emit_outproj(lqh):
                    for ot in range(8):
                        ps = xpool.tile([128, 512], f32, tag="xpool", name=f"psy{ot}")
                        for dc in range(4):
                            nc.tensor.matmul(
                                ps[:],
                                wo_sb[:, 1024 * dc + 128 * ot : 1024 * dc + 128 * ot + 128],
                                xf_sb[:, 1024 * dc + 512 * lqh : 1024 * dc + 512 * lqh + 512],
                                start=(dc == 0),
                                stop=(dc == 3),
                            )
                        y = ysb.tile([128, 512], bf16, tag="ysb")
                        nc.vector.tensor_copy(out=y[:], in_=ps[:])
                        nc.sync.dma_start(
                            out=yT[128 * ot : 128 * ot + 128, 512 * lqh : 512 * lqh + 512],
                            in_=y[:],
                        )

                for lqh in range(2):
                    for dc in range(4):
                        pv_iter(lqh, dc, use_spool=(lqh == 1 and dc == 3))
                # outproj(0) here - it has no dependency on the last normalize
                # chain, so the PE stays busy while that chain drains.
                emit_outproj(0)
                emit_outproj(1)

    nc.compile()
    return nc


def _get_program(lkp=LKP):
    if lkp not in _CACHE:
        _CACHE[lkp] = _build_program(lkp)
    return _CACHE[lkp]


def _bf16(arr):
    import ml_dtypes

    return np.ascontiguousarray(np.asarray(arr, dtype=np.float32).astype(ml_dtypes.bfloat16))


def _pack_chunks(mat_t, nk, L):
    """[nk*128, L] -> [128, nk*L] with chunk k at cols [k*L, (k+1)*L)."""
    return mat_t.reshape(nk, 128, L).transpose(1, 0, 2).reshape(128, nk * L)


def _shard_inputs(inputs, lkp=LKP):
    q = np.ascontiguousarray(inputs["query"], dtype=np.float32)
    k = np.ascontiguousarray(inputs["key"], dtype=np.float32)
    v = np.ascontiguousarray(inputs["value"], dtype=np.float32)
    wr = np.ascontiguousarray(inputs["weak_rela"], dtype=np.float32)
    mask = np.asarray(inputs["mask"])

    def t_weight(W, hsl):
        # Wt [D, HD] -> [128, 4096] with col = dc*1024 + k*128 + c
        Wt = np.asarray(W, dtype=np.float32)[hsl, :].T
        return Wt.reshape(8, 128, 4, 128).transpose(1, 2, 0, 3).reshape(128, 4096)

    def n_weight(W, hsl):
        # Wt [D, HD] -> [128, 4096] with col = k*512 + c
        Wt = np.asarray(W, dtype=np.float32)[hsl, :].T
        return _pack_chunks(Wt, 8, 512)

    in_maps = []
    for c in range(N_CORES):
        b, hh = divmod(c, 2)
        hsl = slice(HD * hh, HD * hh + HD)
        idx = np.nonzero(mask[b, 0])[0]
        nv = len(idx)
        assert nv <= lkp
        pidx = np.concatenate([idx, np.zeros(lkp - nv, dtype=idx.dtype)])
        bias = np.full(lkp, -1.0e9, np.float32)
        bias[:nv] = 0.0
        mb = np.ascontiguousarray(bias.reshape(lkp // 128, 128).T)
        kc, vc, wrc = k[b][pidx], v[b][pidx], wr[b][pidx]
        # wo: [HD, D] -> [128, 4096] with col = dc*1024 + c
        woT = np.asarray(inputs["Wo"], dtype=np.float32)[:, hsl].T
        wo_p = _pack_chunks(woT, 4, 1024)
        m = {
            "xq": _bf16(_pack_chunks(q[b].T, 8, LQ)),
            "xk": _bf16(_pack_chunks(kc.T, 8, lkp)),
            "xr": _bf16(_pack_chunks(wrc.T, 8, lkp)),
            "xv": _bf16(_pack_chunks(vc.T, 8, lkp)),
            "wq": _bf16(t_weight(inputs["Wq"], hsl)),
            "wk": _bf16(t_weight(inputs["Wk"], hsl)),
            "wrk": _bf16(t_weight(inputs["Wrk"], hsl)),
            "wv": _bf16(n_weight(inputs["Wv"], hsl)),
            "wrv": _bf16(n_weight(inputs["Wrv"], hsl)),
            "wo": _bf16(wo_p),
            "bq_pc": np.asarray(inputs["bq"][hsl]).reshape(4, 128).T.astype(np.float32),
            "bk_pc": np.asarray(inputs["bk"][hsl]).reshape(4, 128).T.astype(np.float32),
            "brk_pc": np.asarray(inputs["brk"][hsl]).reshape(4, 128).T.astype(np.float32),
            "bv_bc": np.broadcast_to(inputs["bv"][hsl], (128, HD)).astype(np.float32),
            "brv_bc": np.broadcast_to(inputs["brv"][hsl], (128, HD)).astype(np.float32),
            "maskb": mb,
        }
        in_maps.append({k2: np.ascontiguousarray(v2) for k2, v2 in m.items()})
    return in_maps


def run_on_hw(inputs, trace=False, **kw):
    from concourse.bass_utils import run_bass_kernel_spmd

    mask = np.asarray(inputs["mask"])
    max_valid = max(int(mask[b, 0].sum()) for b in range(B))
    lkp = max(LKP, ((max_valid + 127) // 128) * 128)
    nc = _get_program(lkp)
    in_maps = _shard_inputs(inputs, lkp)
    res = run_bass_kernel_spmd(
        nc, in_maps, core_ids=list(range(N_CORES)), trace=trace, **kw
    )
    bo = np.asarray(inputs["bo"], dtype=np.float32)
    outs = []
    for b in range(B):
        yt = res.results[2 * b]["yT"].astype(np.float32) + res.results[
            2 * b + 1
        ]["yT"].astype(np.float32)
        outs.append(yt.T + bo)
    out = np.stack(outs).astype(np.float32)
    return out, res


def kernel(**inputs):
    out, _ = run_on_hw(inputs)
    return out


# revision 10
# speedup vs baseline: 1.0596x; 1.0460x over previous
"""Fused multi-head cross-attention with relation branch, sharded over 8 NeuronCores.

Sharding: data-parallel over batch (4) x tensor-parallel over head halves (2).
Core c handles batch c//2, heads [8*(c%2), 8*(c%2)+8). Each core computes its
partial output projection; the host sums the two partials per batch and adds bo.

v2 structure (vs v1):
  - Host pre-packs every input into its device SBUF layout ([128, N] tiles),
    so each tensor loads with ONE large DMA descriptor (packets round-robin
    over all 16 DMA engines, so big descriptors lose no bandwidth). Issue is
    split across the two HWDGE queues (sync + scalar) plus gpsimd for the
    small persistent tensors, with critical-path tensors first.
  - dc-major phase 1: for each dim-chunk dc, project q/k/rk then immediately
    emit that chunk's scores+exp, so the ACT engine (co-critical at ~74us)
    starts ~15us earlier than a tensor-major ordering.
  - Scores for one (lqh, dc, m) are 4 matmuls (2 branches x 2 row-tiled K=64
    head matmuls) into one [128, 2048] PSUM tile, exp'd by a single ACTIVATE
    (the mask bias is shared by both branches).
  - Softmax normalize chain runs in bf16 (2x DVE rate): PSUM->SBUF copies,
    DRAM-roundtrip batched reciprocal, gpsimd partition-broadcasts, fma.
  - The last PV iteration's accumulators live in spool (free after the last
    exp) so the output projection for lqh=0 - emitted AFTER all PV work -
    can run on xpool banks while the final normalize chain drains.
  - yT is written bf16 (host upcasts, sums partials, adds bo in f32).
"""

import math

import numpy as np

B, LQ, LK, D, H = 4, 1024, 1024, 1024, 16
DK = D // H
SCALE = 1.0 / math.sqrt(DK)
N_CORES = 8
HD = D // 2  # local dims per core (8 heads * 64)
# Keys are compacted host-side: only unmasked keys are shipped (padded to LKP
# with dummy rows whose mask bias is -1e9, so exp()=0 -> exact same math).
LKP = 640
NM = LKP // 128  # lk chunks

_CACHE = {}


def _build_program(lkp=LKP):
    import concourse.bacc as bacc
    import concourse.mybir as mybir
    import concourse.tile as tile

    LKP = lkp
    NM = LKP // 128

    f32 = mybir.dt.float32
    bf16 = mybir.dt.bfloat16
    Exp = mybir.ActivationFunctionType.Exp
    Copy = mybir.ActivationFunctionType.Copy
    Add = mybir.AluOpType.add
    Mult = mybir.AluOpType.mult

    nc = bacc.Bacc(
        "TRN2",
        target_bir_lowering=False,
        debug=False,
        enable_asserts=False,
        num_devices=N_CORES,
    )

    # DRAM I/O. Everything arrives pre-packed in its SBUF layout.
    # x tensors: [128, 8*L] with k-chunk k at cols [k*L, (k+1)*L).
    xq_d = nc.dram_tensor("xq", [128, 8 * LQ], bf16, kind="ExternalInput").ap()
    xk_d = nc.dram_tensor("xk", [128, 8 * LKP], bf16, kind="ExternalInput").ap()
    xr_d = nc.dram_tensor("xr", [128, 8 * LKP], bf16, kind="ExternalInput").ap()
    xv_d = nc.dram_tensor("xv", [128, 8 * LKP], bf16, kind="ExternalInput").ap()
    # transposed-proj weights: [128, 4*8*128]: col = dc*1024 + k*128 + c
    wq_d = nc.dram_tensor("wq", [128, 4096], bf16, kind="ExternalInput").ap()
    wk_d = nc.dram_tensor("wk", [128, 4096], bf16, kind="ExternalInput").ap()
    wrk_d = nc.dram_tensor("wrk", [128, 4096], bf16, kind="ExternalInput").ap()
    # natural-proj weights: [128, 8*512]: col = k*512 + c
    wv_d = nc.dram_tensor("wv", [128, 4096], bf16, kind="ExternalInput").ap()
    wrv_d = nc.dram_tensor("wrv", [128, 4096], bf16, kind="ExternalInput").ap()
    # output-proj weights: [128, 4*1024]: col = dc*1024 + c
    wo_d = nc.dram_tensor("wo", [128, 4096], bf16, kind="ExternalInput").ap()
    bq_pc = nc.dram_tensor("bq_pc", [128, 4], f32, kind="ExternalInput").ap()
    bk_pc = nc.dram_tensor("bk_pc", [128, 4], f32, kind="ExternalInput").ap()
    brk_pc = nc.dram_tensor("brk_pc", [128, 4], f32, kind="ExternalInput").ap()
    bv_bc = nc.dram_tensor("bv_bc", [128, HD], f32, kind="ExternalInput").ap()
    brv_bc = nc.dram_tensor("brv_bc", [128, HD], f32, kind="ExternalInput").ap()
    maskb = nc.dram_tensor("maskb", [128, NM], f32, kind="ExternalInput").ap()
    yT = nc.dram_tensor("yT", [D, LQ], bf16, kind="ExternalOutput").ap()
    scr1 = nc.dram_tensor("scr1", [8, 2048], bf16, kind="Internal").ap()
    scr2 = nc.dram_tensor("scr2", [8, 2048], bf16, kind="Internal").ap()

    with nc.allow_low_precision(
        reason="bf16 normalize pipeline by design; matmul PSUM acc stays f32"
    ), tile.TileContext(nc) as tc:
        from contextlib import ExitStack

        with ExitStack() as ctx:
            # ---- persistent SBUF tensors (whole-program lifetime) ----
            persist = ctx.enter_context(tc.tile_pool(name="persist", bufs=1))
            qT_sb = persist.tile([128, 4 * LQ], bf16, tag="qT")
            kT_sb = persist.tile([128, 4 * LKP], bf16, tag="kT")
            rkT_sb = persist.tile([128, 4 * LKP], bf16, tag="rkT")
            v_sb = persist.tile([128, NM * 8 * 65], bf16, tag="v")
            rv_sb = persist.tile([128, NM * 8 * 65], bf16, tag="rv")
            xf_sb = persist.tile([128, 4 * LQ], bf16, tag="xf")
            wo_sb = persist.tile([128, 4096], bf16, tag="wo")
            maskb_sb = persist.tile([128, NM], f32, tag="maskb")
            bq_sb = persist.tile([128, 4], f32, tag="bq")
            bk_sb = persist.tile([128, 4], f32, tag="bk")
            brk_sb = persist.tile([128, 4], f32, tag="brk")
            bv_sb = persist.tile([128, HD], f32, tag="bv")
            brv_sb = persist.tile([128, HD], f32, tag="brv")

            v4 = v_sb[:].rearrange("p (m h c) -> p m h c", m=NM, h=8, c=65)
            rv4 = rv_sb[:].rearrange("p (m h c) -> p m h c", m=NM, h=8, c=65)
            nc.vector.memset(v4[:, :, :, 64:65], 1.0)
            nc.vector.memset(rv4[:, :, :, 64:65], 1.0)

            # score PSUM (4 banks) + p-tile pool; 16 quad bufs of lookahead
            spool = ctx.enter_context(tc.tile_pool(name="spool", bufs=1, space="PSUM"))
            ppool = ctx.enter_context(tc.tile_pool(name="ppool", bufs=16))
            p_tiles = {}

            def emit_score_quad(lqh, dc, m):
                qsl = slice(1024 * dc + 512 * lqh, 1024 * dc + 512 * lqh + 512)
                ksl = slice(LKP * dc + 128 * m, LKP * dc + 128 * m + 128)
                s = spool.tile([128, 2048], f32, tag="spool", name="s")
                nc.tensor.matmul(s[:, 0:512], kT_sb[0:64, ksl], qT_sb[0:64, qsl])
                nc.tensor.matmul(
                    s[:, 512:1024], kT_sb[64:128, ksl], qT_sb[64:128, qsl]
                )
                nc.tensor.matmul(
                    s[:, 1024:1536], rkT_sb[0:64, ksl], qT_sb[0:64, qsl]
                )
                nc.tensor.matmul(
                    s[:, 1536:2048], rkT_sb[64:128, ksl], qT_sb[64:128, qsl]
                )
                p = ppool.tile([128, 2048], bf16, tag="ppool", name="p")
                nc.scalar.activation(
                    p[:], s[:], Exp, bias=maskb_sb[:, m : m + 1], scale=SCALE
                )
                p_tiles[(lqh, dc, m)] = p

            # ---------------- Phase 1: dc-major q/k/rk + scores(0) ----------
            with ExitStack() as ph1:
                ppsum = ph1.enter_context(
                    tc.tile_pool(name="ppsum", bufs=2, space="PSUM")
                )
                # input tensors live only through phase 1; phase-2 pools
                # reuse their SBUF space
                xin = ph1.enter_context(tc.tile_pool(name="xin", bufs=1))
                xq_sb = xin.tile([128, 8 * LQ], bf16, tag="xq")
                xk_sb = xin.tile([128, 8 * LKP], bf16, tag="xk")
                xr_sb = xin.tile([128, 8 * LKP], bf16, tag="xr")
                xv_sb = xin.tile([128, 8 * LKP], bf16, tag="xv")
                wq_sb = xin.tile([128, 4096], bf16, tag="wq")
                wk_sb = xin.tile([128, 4096], bf16, tag="wk")
                wrk_sb = xin.tile([128, 4096], bf16, tag="wrk")
                wv_sb = xin.tile([128, 4096], bf16, tag="wv")
                wrv_sb = xin.tile([128, 4096], bf16, tag="wrv")

                # ---- input DMAs ----
                # A single in-flight descriptor stream tops out well below the
                # 358GB/s aggregate, so each big tensor is split into quarters
                # interleaved across both HWDGE queues (sync + scalar), in
                # consumption order: xq/wq(dc0) -> xk/wk(dc0) -> xr/wrk(dc0)
                # -> remaining weights -> v-branch inputs.
                def split4(dst, src, n):
                    q = n // 4
                    for j in range(4):
                        eng = nc.sync if j % 2 == 0 else nc.scalar
                        eng.dma_start(
                            out=dst[:, j * q : (j + 1) * q],
                            in_=src[:, j * q : (j + 1) * q],
                        )

                split4(xq_sb, xq_d, 8 * LQ)
                nc.sync.dma_start(out=wq_sb[:, 0:1024], in_=wq_d[:, 0:1024])
                split4(xk_sb, xk_d, 8 * LKP)
                nc.scalar.dma_start(out=wk_sb[:, 0:1024], in_=wk_d[:, 0:1024])
                split4(xr_sb, xr_d, 8 * LKP)
                nc.sync.dma_start(out=wrk_sb[:, 0:1024], in_=wrk_d[:, 0:1024])
                nc.scalar.dma_start(out=wq_sb[:, 1024:4096], in_=wq_d[:, 1024:4096])
                nc.sync.dma_start(out=wk_sb[:, 1024:4096], in_=wk_d[:, 1024:4096])
                nc.scalar.dma_start(out=wrk_sb[:, 1024:4096], in_=wrk_d[:, 1024:4096])
                split4(xv_sb, xv_d, 8 * LKP)
                nc.sync.dma_start(out=wv_sb[:], in_=wv_d)
                nc.scalar.dma_start(out=wrv_sb[:], in_=wrv_d)
                nc.sync.dma_start(out=wo_sb[:], in_=wo_d)
                nc.scalar.dma_start(out=bv_sb[:], in_=bv_bc)
                nc.sync.dma_start(out=brv_sb[:], in_=brv_bc)
                # gpsimd (software DGE): tiny persistent tensors
                nc.gpsimd.dma_start(out=maskb_sb[:], in_=maskb)
                nc.gpsimd.dma_start(out=bq_sb[:], in_=bq_pc)
                nc.gpsimd.dma_start(out=bk_sb[:], in_=bk_pc)
                nc.gpsimd.dma_start(out=brk_sb[:], in_=brk_pc)

                def tproj(dc, x_sb, w_sb, b_sb, out_sb, LL):
                    nsl = [slice(a, min(a + 512, LL)) for a in range(0, LL, 512)]
                    ps = ppsum.tile([128, LL], f32, tag="ppsum")
                    for k in range(8):
                        wsl = w_sb[:, 1024 * dc + 128 * k : 1024 * dc + 128 * k + 128]
                        for sl in nsl:
                            nc.tensor.matmul(
                                ps[:, sl],
                                wsl,
                                x_sb[:, LL * k + sl.start : LL * k + sl.stop],
                                start=(k == 0),
                                stop=(k == 7),
                            )
                    nc.vector.tensor_scalar(
                        out=out_sb[:, LL * dc : LL * dc + LL],
                        in0=ps[:],
                        scalar1=b_sb[:, dc : dc + 1],
                        scalar2=None,
                        op0=Add,
                    )

                for dc in range(4):
                    tproj(dc, xq_sb, wq_sb, bq_sb, qT_sb, LQ)
                    tproj(dc, xk_sb, wk_sb, bk_sb, kT_sb, LKP)
                    tproj(dc, xr_sb, wrk_sb, brk_sb, rkT_sb, LKP)
                    for m in range(NM):
                        emit_score_quad(0, dc, m)

                # scores for the second lq half (exp keeps the ACT engine fed
                # while the v/rv projections run on PE)
                for dc in range(4):
                    for m in range(NM):
                        emit_score_quad(1, dc, m)

                # Natural-orientation projections for v / rv.
                for x_sb, w_sb, b_sb, out4 in (
                    (xv_sb, wv_sb, bv_sb, v4),
                    (xr_sb, wrv_sb, brv_sb, rv4),
                ):
                    for m in range(NM):
                        ps = ppsum.tile([128, 512], f32, tag="ppsum")
                        for k in range(8):
                            nc.tensor.matmul(
                                ps[:],
                                x_sb[:, LKP * k + 128 * m : LKP * k + 128 * m + 128],
                                w_sb[:, 512 * k : 512 * k + 512],
                                start=(k == 0),
                                stop=(k == 7),
                            )
                        nc.vector.tensor_tensor(
                            out=out4[:, m, :, 0:64],
                            in0=ps[:].rearrange("p (h c) -> p h c", h=8, c=64),
                            in1=b_sb[:].rearrange("p (h c) -> p h c", h=8, c=64),
                            op=Add,
                        )

            # -------- Phase 2: PV accumulation, normalize, output projection --
            # The normalize chain is software-pipelined one iteration deep so
            # its DMA round-trip latency never blocks the in-order DVE queue:
            #   front(it): PV matmuls + PSUM->SBUF evictions + Z-row DMAs
            #   finish(it-1): broadcast-multiply + add -> xf
            #   mid(it): reciprocal + scatter + partition-broadcast issues
            with ExitStack() as ph2:
                xpool = ph2.enter_context(
                    tc.tile_pool(name="xpool", bufs=4, space="PSUM")
                )
                xsb = ph2.enter_context(tc.tile_pool(name="xsb", bufs=6))
                sgp = ph2.enter_context(tc.tile_pool(name="sgp", bufs=2))
                bcp = ph2.enter_context(tc.tile_pool(name="bcp", bufs=8))
                ysb = ph2.enter_context(tc.tile_pool(name="ysb", bufs=4))

                def pv_front(lqh, dc, use_spool):
                    it = 2 * dc + lqh
                    xacc = {}
                    if use_spool:
                        w2 = spool.tile([128, 2048], f32, tag="spool", name="xaccs")
                        for j, (br, hs) in enumerate(
                            [(0, 0), (0, 1), (1, 0), (1, 1)]
                        ):
                            xacc[(br, hs)] = w2[:, 512 * j : 512 * j + 512]
                    else:
                        for br in range(2):
                            for hs in range(2):
                                xacc[(br, hs)] = xpool.tile(
                                    [65, 512], f32, tag="xpool", name=f"xacc{br}{hs}"
                                )[:, :]
                    for m in range(NM):
                        pt = p_tiles[(lqh, dc, m)]
                        for br, vv in ((0, v4), (1, rv4)):
                            for hs in range(2):
                                nc.tensor.matmul(
                                    xacc[(br, hs)][0:65, :],
                                    vv[:, m, 2 * dc + hs, :],
                                    pt[:, 1024 * br + 512 * hs : 1024 * br + 512 * hs + 512],
                                    start=(m == 0),
                                    stop=(m == NM - 1),
                                )
                    # Evict accumulators (bf16) so the PSUM banks free fast;
                    # row 64 carries the softmax denominators.
                    xs_all = xsb.tile([65, 2048], bf16, tag="xsall", bufs=3)
                    xs = {}
                    for j, (br, hs) in enumerate([(0, 0), (1, 0), (0, 1), (1, 1)]):
                        sl = xs_all[:, 512 * j : 512 * j + 512]
                        nc.vector.tensor_copy(out=sl, in_=xacc[(br, hs)][0:65, :])
                        xs[(br, hs)] = sl
                    sg = sgp.tile([128, 16], bf16, tag="sgp")
                    nc.sync.dma_start(out=scr1[it, :], in_=xs_all[64:65, :])
                    nc.sync.dma_start(out=sg[:], in_=scr1[it, :])
                    return {"it": it, "lqh": lqh, "dc": dc, "xs": xs, "sg": sg}

                def pv_mid(st):
                    # Batch-reciprocate the 4 denominator rows via DRAM (the
                    # [1,2048] row respreads over 128 DVE lanes as [128,16]).
                    it = st["it"]
                    nc.vector.reciprocal(st["sg"][:], st["sg"][:])
                    nc.sync.dma_start(out=scr2[it, :], in_=st["sg"][:])
                    bc = {}
                    for j in range(4):
                        t = bcp.tile([64, 512], bf16, tag="bcp", name=f"bc{j}")
                        nc.gpsimd.dma_start(
                            out=t[:],
                            in_=scr2[it : it + 1, 512 * j : 512 * j + 512]
                            .partition_broadcast(64)[:, 0, :],
                        )
                        bc[j] = t
                    st["bc"] = bc

                def pv_finish(st):
                    lqh, dc, xs, bc = st["lqh"], st["dc"], st["xs"], st["bc"]
                    for hs in range(2):
                        jv, jr = 2 * hs, 2 * hs + 1
                        t1 = xsb.tile([65, 512], bf16, tag="xsb")
                        nc.vector.tensor_tensor(
                            out=t1[0:64, :], in0=xs[(0, hs)][0:64, :],
                            in1=bc[jv][:], op=Mult,
                        )
                        t2 = xsb.tile([65, 512], bf16, tag="xsb")
                        nc.vector.tensor_tensor(
                            out=t2[0:64, :], in0=xs[(1, hs)][0:64, :],
                            in1=bc[jr][:], op=Mult,
                        )
                        xf_slice = slice(
                            1024 * dc + 512 * lqh, 1024 * dc + 512 * lqh + 512
                        )
                        if hs == 0:
                            nc.vector.tensor_tensor(
                                out=xf_sb[0:64, xf_slice], in0=t1[0:64, :],
                                in1=t2[0:64, :], op=Add,
                            )
                        else:
                            t3 = xsb.tile([65, 512], bf16, tag="xsb")
                            nc.vector.tensor_tensor(
                                out=t3[0:64, :], in0=t1[0:64, :], in1=t2[0:64, :],
                                op=Add,
                            )
                            nc.sync.dma_start(
                                out=xf_sb[64:128, xf_slice], in_=t3[0:64, :]
                            )

                def y_evict(ps, ot, lqh, qno):
                    # PSUM->SBUF eviction on the ACT engine (idle by now), DMA
                    # alternating between the two HWDGE queues.
                    y = ysb.tile([128, 512], bf16, tag="ysb")
                    nc.scalar.activation(y[:], ps[:], Copy)
                    eng = nc.sync if qno % 2 == 0 else nc.scalar
                    eng.dma_start(
                        out=yT[128 * ot : 128 * ot + 128, 512 * lqh : 512 * lqh + 512],
                        in_=y[:],
                    )

                def emit_outproj0():
                    for ot in range(8):
                        ps = xpool.tile([128, 512], f32, tag="xpool", name=f"psy{ot}")
                        for dc in range(4):
                            nc.tensor.matmul(
                                ps[:],
                                wo_sb[:, 1024 * dc + 128 * ot : 1024 * dc + 128 * ot + 128],
                                xf_sb[:, 1024 * dc : 1024 * dc + 512],
                                start=(dc == 0),
                                stop=(dc == 3),
                            )
                        y_evict(ps, ot, 0, ot)

                def emit_outproj1():
                    # dc-outer with all 8 accumulators live (4 xpool banks + 4
                    # spool quarters): only the final dc=3 batch of matmuls
                    # waits on the last normalize chain.
                    pss = []
                    for i in range(4):
                        pss.append(
                            xpool.tile([128, 512], f32, tag="xpool", name=f"psw{i}")
                        )
                    w2 = spool.tile([128, 2048], f32, tag="spool", name="psw2")
                    for i in range(4):
                        pss.append(w2[:, 512 * i : 512 * i + 512])
                    for dc in range(4):
                        for ot in range(8):
                            nc.tensor.matmul(
                                pss[ot][0:128, :],
                                wo_sb[:, 1024 * dc + 128 * ot : 1024 * dc + 128 * ot + 128],
                                xf_sb[:, 1024 * dc + 512 : 1024 * dc + 1024],
                                start=(dc == 0),
                                stop=(dc == 3),
                            )
                    for ot in range(8):
                        y_evict(pss[ot], ot, 1, ot + 1)

                prev = None
                for lqh in range(2):
                    for dc in range(4):
                        st = pv_front(lqh, dc, use_spool=(lqh == 1 and dc == 3))
                        if prev is not None:
                            pv_finish(prev)
                        pv_mid(st)
                        prev = st
                # outproj(0) has no dependency on the last normalize chain, so
                # the PE stays busy while that chain drains.
                emit_outproj0()
                pv_finish(prev)
                emit_outproj1()

    nc.compile()
    return nc


def _get_program(lkp=LKP):
    if lkp not in _CACHE:
        _CACHE[lkp] = _build_program(lkp)
    return _CACHE[lkp]


def _bf16(arr):
    import ml_dtypes

    return np.ascontiguousarray(np.asarray(arr, dtype=np.float32).astype(ml_dtypes.bfloat16))


def _pack_chunks(mat_t, nk, L):
    """[nk*128, L] -> [128, nk*L] with chunk k at cols [k*L, (k+1)*L)."""
    return mat_t.reshape(nk, 128, L).transpose(1, 0, 2).reshape(128, nk * L)


def _shard_inputs(inputs, lkp=LKP):
    q = np.ascontiguousarray(inputs["query"], dtype=np.float32)
    k = np.ascontiguousarray(inputs["key"], dtype=np.float32)
    v = np.ascontiguousarray(inputs["value"], dtype=np.float32)
    wr = np.ascontiguousarray(inputs["weak_rela"], dtype=np.float32)
    mask = np.asarray(inputs["mask"])

    def t_weight(W, hsl):
        # Wt [D, HD] -> [128, 4096] with col = dc*1024 + k*128 + c
        Wt = np.asarray(W, dtype=np.float32)[hsl, :].T
        return Wt.reshape(8, 128, 4, 128).transpose(1, 2, 0, 3).reshape(128, 4096)

    def n_weight(W, hsl):
        # Wt [D, HD] -> [128, 4096] with col = k*512 + c
        Wt = np.asarray(W, dtype=np.float32)[hsl, :].T
        return _pack_chunks(Wt, 8, 512)

    in_maps = []
    for c in range(N_CORES):
        b, hh = divmod(c, 2)
        hsl = slice(HD * hh, HD * hh + HD)
        idx = np.nonzero(mask[b, 0])[0]
        nv = len(idx)
        assert nv <= lkp
        pidx = np.concatenate([idx, np.zeros(lkp - nv, dtype=idx.dtype)])
        bias = np.full(lkp, -1.0e9, np.float32)
        bias[:nv] = 0.0
        mb = np.ascontiguousarray(bias.reshape(lkp // 128, 128).T)
        kc, vc, wrc = k[b][pidx], v[b][pidx], wr[b][pidx]
        # wo: [HD, D] -> [128, 4096] with col = dc*1024 + c
        woT = np.asarray(inputs["Wo"], dtype=np.float32)[:, hsl].T
        wo_p = _pack_chunks(woT, 4, 1024)
        m = {
            "xq": _bf16(_pack_chunks(q[b].T, 8, LQ)),
            "xk": _bf16(_pack_chunks(kc.T, 8, lkp)),
            "xr": _bf16(_pack_chunks(wrc.T, 8, lkp)),
            "xv": _bf16(_pack_chunks(vc.T, 8, lkp)),
            "wq": _bf16(t_weight(inputs["Wq"], hsl)),
            "wk": _bf16(t_weight(inputs["Wk"], hsl)),
            "wrk": _bf16(t_weight(inputs["Wrk"], hsl)),
            "wv": _bf16(n_weight(inputs["Wv"], hsl)),
            "wrv": _bf16(n_weight(inputs["Wrv"], hsl)),
            "wo": _bf16(wo_p),
            "bq_pc": np.asarray(inputs["bq"][hsl]).reshape(4, 128).T.astype(np.float32),
            "bk_pc": np.asarray(inputs["bk"][hsl]).reshape(4, 128).T.astype(np.float32),
            "brk_pc": np.asarray(inputs["brk"][hsl]).reshape(4, 128).T.astype(np.float32),
            "bv_bc": np.broadcast_to(inputs["bv"][hsl], (128, HD)).astype(np.float32),
            "brv_bc": np.broadcast_to(inputs["brv"][hsl], (128, HD)).astype(np.float32),
            "maskb": mb,
        }
        in_maps.append({k2: np.ascontiguousarray(v2) for k2, v2 in m.items()})
    return in_maps


def run_on_hw(inputs, trace=False, **kw):
    from concourse.bass_utils import run_bass_kernel_spmd

    mask = np.asarray(inputs["mask"])
    max_valid = max(int(mask[b, 0].sum()) for b in range(B))
    lkp = max(LKP, ((max_valid + 127) // 128) * 128)
    nc = _get_program(lkp)
    in_maps = _shard_inputs(inputs, lkp)
    res = run_bass_kernel_spmd(
        nc, in_maps, core_ids=list(range(N_CORES)), trace=trace, **kw
    )
    bo = np.asarray(inputs["bo"], dtype=np.float32)
    outs = []
    for b in range(B):
        yt = res.results[2 * b]["yT"].astype(np.float32) + res.results[
            2 * b + 1
        ]["yT"].astype(np.float32)
        outs.append(yt.T + bo)
    out = np.stack(outs).astype(np.float32)
    return out, res


def kernel(**inputs):
    out, _ = run_on_hw(inputs)
    return out


# revision 15
# speedup vs baseline: 1.1093x; 1.0469x over previous
"""Fused multi-head cross-attention with relation branch, sharded over 8 NeuronCores.

Sharding: data-parallel over batch (4) x tensor-parallel over head halves (2).
Core c handles batch c//2, heads [8*(c%2), 8*(c%2)+8). Each core computes its
partial output projection; the host sums the two partials per batch and adds bo.

v2 structure (vs v1):
  - Host pre-packs every input into its device SBUF layout ([128, N] tiles),
    so each tensor loads with ONE large DMA descriptor (packets round-robin
    over all 16 DMA engines, so big descriptors lose no bandwidth). Issue is
    split across the two HWDGE queues (sync + scalar) plus gpsimd for the
    small persistent tensors, with critical-path tensors first.
  - dc-major phase 1: for each dim-chunk dc, project q/k/rk then immediately
    emit that chunk's scores+exp, so the ACT engine (co-critical at ~74us)
    starts ~15us earlier than a tensor-major ordering.
  - Scores for one (lqh, dc, m) are 4 matmuls (2 branches x 2 row-tiled K=64
    head matmuls) into one [128, 2048] PSUM tile, exp'd by a single ACTIVATE
    (the mask bias is shared by both branches).
  - Softmax normalize chain runs in bf16 (2x DVE rate): PSUM->SBUF copies,
    DRAM-roundtrip batched reciprocal, gpsimd partition-broadcasts, fma.
  - The last PV iteration's accumulators live in spool (free after the last
    exp) so the output projection for lqh=0 - emitted AFTER all PV work -
    can run on xpool banks while the final normalize chain drains.
  - yT is written bf16 (host upcasts, sums partials, adds bo in f32).
"""

import math

import numpy as np

B, LQ, LK, D, H = 4, 1024, 1024, 1024, 16
DK = D // H
SCALE = 1.0 / math.sqrt(DK)
N_CORES = 8
HD = D // 2  # local dims per core (8 heads * 64)
# Keys are compacted host-side: only unmasked keys are shipped (padded to LKP
# with dummy rows whose mask bias is -1e9, so exp()=0 -> exact same math).
LKP = 640
NM = LKP // 128  # lk chunks

_CACHE = {}


def _build_program(lkp=LKP):
    import concourse.bacc as bacc
    import concourse.mybir as mybir
    import concourse.tile as tile

    LKP = lkp
    NM = LKP // 128

    f32 = mybir.dt.float32
    bf16 = mybir.dt.bfloat16
    Exp = mybir.ActivationFunctionType.Exp
    Copy = mybir.ActivationFunctionType.Copy
    Add = mybir.AluOpType.add
    Mult = mybir.AluOpType.mult

    nc = bacc.Bacc(
        "TRN2",
        target_bir_lowering=False,
        debug=False,
        enable_asserts=False,
        num_devices=N_CORES,
    )

    # DRAM I/O. Everything arrives pre-packed in its SBUF layout.
    # x tensors: [128, 8*L] with k-chunk k at cols [k*L, (k+1)*L).
    xq_d = nc.dram_tensor("xq", [128, 8 * LQ], bf16, kind="ExternalInput").ap()
    xk_d = nc.dram_tensor("xk", [128, 8 * LKP], bf16, kind="ExternalInput").ap()
    xr_d = nc.dram_tensor("xr", [128, 8 * LKP], bf16, kind="ExternalInput").ap()
    xv_d = nc.dram_tensor("xv", [128, 8 * LKP], bf16, kind="ExternalInput").ap()
    # transposed-proj weights: [128, 4*8*128]: col = dc*1024 + k*128 + c
    wq_d = nc.dram_tensor("wq", [128, 4096], bf16, kind="ExternalInput").ap()
    wk_d = nc.dram_tensor("wk", [128, 4096], bf16, kind="ExternalInput").ap()
    wrk_d = nc.dram_tensor("wrk", [128, 4096], bf16, kind="ExternalInput").ap()
    # natural-proj weights: [128, 8*512]: col = k*512 + c
    wv_d = nc.dram_tensor("wv", [128, 4096], bf16, kind="ExternalInput").ap()
    wrv_d = nc.dram_tensor("wrv", [128, 4096], bf16, kind="ExternalInput").ap()
    # output-proj weights: [128, 4*1024]: col = dc*1024 + c
    wo_d = nc.dram_tensor("wo", [128, 4096], bf16, kind="ExternalInput").ap()
    bq_pc = nc.dram_tensor("bq_pc", [128, 4], f32, kind="ExternalInput").ap()
    bk_pc = nc.dram_tensor("bk_pc", [128, 4], f32, kind="ExternalInput").ap()
    brk_pc = nc.dram_tensor("brk_pc", [128, 4], f32, kind="ExternalInput").ap()
    bv_bc = nc.dram_tensor("bv_bc", [128, HD], f32, kind="ExternalInput").ap()
    brv_bc = nc.dram_tensor("brv_bc", [128, HD], f32, kind="ExternalInput").ap()
    maskb = nc.dram_tensor("maskb", [128, NM], f32, kind="ExternalInput").ap()
    yT = nc.dram_tensor("yT", [D, LQ], bf16, kind="ExternalOutput").ap()
    scr1 = nc.dram_tensor("scr1", [8, 2048], bf16, kind="Internal").ap()
    scr2 = nc.dram_tensor("scr2", [8, 2048], bf16, kind="Internal").ap()

    with nc.allow_low_precision(
        reason="bf16 normalize pipeline by design; matmul PSUM acc stays f32"
    ), tile.TileContext(nc) as tc:
        from contextlib import ExitStack

        with ExitStack() as ctx:
            # ---- persistent SBUF tensors (whole-program lifetime) ----
            persist = ctx.enter_context(tc.tile_pool(name="persist", bufs=1))
            qT_sb = persist.tile([128, 4 * LQ], bf16, tag="qT")
            kT_sb = persist.tile([128, 4 * LKP], bf16, tag="kT")
            rkT_sb = persist.tile([128, 4 * LKP], bf16, tag="rkT")
            v_sb = persist.tile([128, NM * 8 * 65], bf16, tag="v")
            rv_sb = persist.tile([128, NM * 8 * 65], bf16, tag="rv")
            xf_sb = persist.tile([128, 4 * LQ], bf16, tag="xf")
            wo_sb = persist.tile([128, 4096], bf16, tag="wo")
            maskb_sb = persist.tile([128, NM], f32, tag="maskb")
            bq_sb = persist.tile([128, 4], f32, tag="bq")
            bk_sb = persist.tile([128, 4], f32, tag="bk")
            brk_sb = persist.tile([128, 4], f32, tag="brk")
            bv_sb = persist.tile([128, HD], f32, tag="bv")
            brv_sb = persist.tile([128, HD], f32, tag="brv")

            v4 = v_sb[:].rearrange("p (m h c) -> p m h c", m=NM, h=8, c=65)
            rv4 = rv_sb[:].rearrange("p (m h c) -> p m h c", m=NM, h=8, c=65)
            nc.vector.memset(v4[:, :, :, 64:65], 1.0)
            nc.vector.memset(rv4[:, :, :, 64:65], 1.0)

            # Score PSUM: 2 bufs x [128,1024] so exp(i) overlaps the score
            # matmuls of tile i+1 (a single buffer serializes the ACT engine
            # behind a semaphore round-trip per tile). ppool = exp lookahead.
            spool = ctx.enter_context(tc.tile_pool(name="spool", bufs=2, space="PSUM"))
            ppool = ctx.enter_context(tc.tile_pool(name="ppool", bufs=32))
            p_tiles = {}

            def emit_score_pair(lqh, dc, m, br, kt):
                qsl = slice(1024 * dc + 512 * lqh, 1024 * dc + 512 * lqh + 512)
                ksl = slice(LKP * dc + 128 * m, LKP * dc + 128 * m + 128)
                s = spool.tile([128, 1024], f32, tag="spool", name="s")
                nc.tensor.matmul(s[:, 0:512], kt[0:64, ksl], qT_sb[0:64, qsl])
                nc.tensor.matmul(s[:, 512:1024], kt[64:128, ksl], qT_sb[64:128, qsl])
                p = ppool.tile([128, 1024], bf16, tag="ppool", name="p")
                nc.scalar.activation(
                    p[:], s[:], Exp, bias=maskb_sb[:, m : m + 1], scale=SCALE
                )
                p_tiles[(lqh, dc, m, br)] = p

            def emit_scores(lqh, dc):
                for m in range(NM):
                    for br, kt in ((0, kT_sb), (1, rkT_sb)):
                        emit_score_pair(lqh, dc, m, br, kt)

            # ---------------- Phase 1: dc-major q/k/rk + scores(0) ----------
            with ExitStack() as ph1:
                ppsum = ph1.enter_context(
                    tc.tile_pool(name="ppsum", bufs=2, space="PSUM")
                )
                # input tensors live only through phase 1; phase-2 pools
                # reuse their SBUF space
                xin = ph1.enter_context(tc.tile_pool(name="xin", bufs=1))
                xq_sb = xin.tile([128, 8 * LQ], bf16, tag="xq")
                xk_sb = xin.tile([128, 8 * LKP], bf16, tag="xk")
                xr_sb = xin.tile([128, 8 * LKP], bf16, tag="xr")
                xv_sb = xin.tile([128, 8 * LKP], bf16, tag="xv")
                wq_sb = xin.tile([128, 4096], bf16, tag="wq")
                wk_sb = xin.tile([128, 4096], bf16, tag="wk")
                wrk_sb = xin.tile([128, 4096], bf16, tag="wrk")
                wv_sb = xin.tile([128, 4096], bf16, tag="wv")
                wrv_sb = xin.tile([128, 4096], bf16, tag="wrv")

                # ---- input DMAs ----
                # A single in-flight descriptor stream tops out well below the
                # 358GB/s aggregate, so each big tensor is split into quarters
                # interleaved across both HWDGE queues (sync + scalar), in
                # consumption order: xq/wq(dc0) -> xk/wk(dc0) -> xr/wrk(dc0)
                # -> remaining weights -> v-branch inputs.
                def split4(dst, src, n):
                    q = n // 4
                    for j in range(4):
                        eng = nc.sync if j % 2 == 0 else nc.scalar
                        eng.dma_start(
                            out=dst[:, j * q : (j + 1) * q],
                            in_=src[:, j * q : (j + 1) * q],
                        )

                split4(xq_sb, xq_d, 8 * LQ)
                nc.sync.dma_start(out=wq_sb[:, 0:1024], in_=wq_d[:, 0:1024])
                split4(xk_sb, xk_d, 8 * LKP)
                nc.scalar.dma_start(out=wk_sb[:, 0:1024], in_=wk_d[:, 0:1024])
                split4(xr_sb, xr_d, 8 * LKP)
                nc.sync.dma_start(out=wrk_sb[:, 0:1024], in_=wrk_d[:, 0:1024])
                nc.scalar.dma_start(out=wq_sb[:, 1024:4096], in_=wq_d[:, 1024:4096])
                nc.sync.dma_start(out=wk_sb[:, 1024:4096], in_=wk_d[:, 1024:4096])
                nc.scalar.dma_start(out=wrk_sb[:, 1024:4096], in_=wrk_d[:, 1024:4096])
                split4(xv_sb, xv_d, 8 * LKP)
                nc.sync.dma_start(out=wv_sb[:], in_=wv_d)
                nc.scalar.dma_start(out=wrv_sb[:], in_=wrv_d)
                nc.sync.dma_start(out=wo_sb[:], in_=wo_d)
                nc.scalar.dma_start(out=bv_sb[:], in_=bv_bc)
                nc.sync.dma_start(out=brv_sb[:], in_=brv_bc)
                # gpsimd (software DGE): tiny persistent tensors
                nc.gpsimd.dma_start(out=maskb_sb[:], in_=maskb)
                nc.gpsimd.dma_start(out=bq_sb[:], in_=bq_pc)
                nc.gpsimd.dma_start(out=bk_sb[:], in_=bk_pc)
                nc.gpsimd.dma_start(out=brk_sb[:], in_=brk_pc)

                def tproj(dc, x_sb, w_sb, b_sb, out_sb, LL):
                    nsl = [slice(a, min(a + 512, LL)) for a in range(0, LL, 512)]
                    ps = ppsum.tile([128, LL], f32, tag="ppsum")
                    for k in range(8):
                        wsl = w_sb[:, 1024 * dc + 128 * k : 1024 * dc + 128 * k + 128]
                        for sl in nsl:
                            nc.tensor.matmul(
                                ps[:, sl],
                                wsl,
                                x_sb[:, LL * k + sl.start : LL * k + sl.stop],
                                start=(k == 0),
                                stop=(k == 7),
                            )
                    nc.vector.tensor_scalar(
                        out=out_sb[:, LL * dc : LL * dc + LL],
                        in0=ps[:],
                        scalar1=b_sb[:, dc : dc + 1],
                        scalar2=None,
                        op0=Add,
                    )

                for dc in range(4):
                    tproj(dc, xq_sb, wq_sb, bq_sb, qT_sb, LQ)
                    tproj(dc, xk_sb, wk_sb, bk_sb, kT_sb, LKP)
                    tproj(dc, xr_sb, wrk_sb, brk_sb, rkT_sb, LKP)
                    emit_scores(0, dc)

                # scores for the second lq half (exp keeps the ACT engine fed
                # while the v/rv projections run on PE)
                for dc in range(4):
                    emit_scores(1, dc)

                # Natural-orientation projections for v / rv.
                for x_sb, w_sb, b_sb, out4 in (
                    (xv_sb, wv_sb, bv_sb, v4),
                    (xr_sb, wrv_sb, brv_sb, rv4),
                ):
                    for m in range(NM):
                        ps = ppsum.tile([128, 512], f32, tag="ppsum")
                        for k in range(8):
                            nc.tensor.matmul(
                                ps[:],
                                x_sb[:, LKP * k + 128 * m : LKP * k + 128 * m + 128],
                                w_sb[:, 512 * k : 512 * k + 512],
                                start=(k == 0),
                                stop=(k == 7),
                            )
                        nc.vector.tensor_tensor(
                            out=out4[:, m, :, 0:64],
                            in0=ps[:].rearrange("p (h c) -> p h c", h=8, c=64),
                            in1=b_sb[:].rearrange("p (h c) -> p h c", h=8, c=64),
                            op=Add,
                        )

            # -------- Phase 2: PV accumulation, normalize, output projection --
            # The normalize chain is software-pipelined one iteration deep so
            # its DMA round-trip latency never blocks the in-order DVE queue:
            #   front(it): PV matmuls + PSUM->SBUF evictions + Z-row DMAs
            #   finish(it-1): broadcast-multiply + add -> xf
            #   mid(it): reciprocal + scatter + partition-broadcast issues
            with ExitStack() as ph2:
                xpool = ph2.enter_context(
                    tc.tile_pool(name="xpool", bufs=4, space="PSUM")
                )
                xsb = ph2.enter_context(tc.tile_pool(name="xsb", bufs=6))
                sgp = ph2.enter_context(tc.tile_pool(name="sgp", bufs=2))
                bcp = ph2.enter_context(tc.tile_pool(name="bcp", bufs=8))
                ysb = ph2.enter_context(tc.tile_pool(name="ysb", bufs=4))

                def pv_front(lqh, dc, use_spool, last=False):
                    it = 2 * dc + lqh
                    xacc = {}
                    if use_spool:
                        w2a = spool.tile([128, 1024], f32, tag="spool", name="xacca")
                        w2b = spool.tile([128, 1024], f32, tag="spool", name="xaccb")
                        for j, (br, hs) in enumerate(
                            [(0, 0), (0, 1), (1, 0), (1, 1)]
                        ):
                            w2 = w2a if j < 2 else w2b
                            xacc[(br, hs)] = w2[:, 512 * (j % 2) : 512 * (j % 2) + 512]
                    else:
                        for br in range(2):
                            for hs in range(2):
                                xacc[(br, hs)] = xpool.tile(
                                    [65, 512], f32, tag="xpool", name=f"xacc{br}{hs}"
                                )[:, :]
                    for m in range(NM):
                        for br, vv in ((0, v4), (1, rv4)):
                            pt = p_tiles[(lqh, dc, m, br)]
                            for hs in range(2):
                                nc.tensor.matmul(
                                    xacc[(br, hs)][0:65, :],
                                    vv[:, m, 2 * dc + hs, :],
                                    pt[:, 512 * hs : 512 * hs + 512],
                                    start=(m == 0),
                                    stop=(m == NM - 1),
                                )
                    # Evict accumulators (bf16) so the PSUM banks free fast;
                    # row 64 carries the softmax denominators. For the last
                    # iteration the chain is latency-critical: exps are done,
                    # so half the copies go to the idle ACT engine.
                    xs_all = xsb.tile([65, 2048], bf16, tag="xsall", bufs=3)
                    xs = {}
                    for j, (br, hs) in enumerate([(0, 0), (1, 0), (0, 1), (1, 1)]):
                        sl = xs_all[:, 512 * j : 512 * j + 512]
                        if last and j >= 2:
                            nc.scalar.activation(sl, xacc[(br, hs)][0:65, :], Copy)
                        else:
                            nc.vector.tensor_copy(out=sl, in_=xacc[(br, hs)][0:65, :])
                        xs[(br, hs)] = sl
                    # Respread the [1,2048] denominator row over 128 DVE lanes
                    # with an SBUF->SBUF DMA (no DRAM round-trip).
                    sg = sgp.tile([128, 16], bf16, tag="sgp")
                    nc.sync.dma_start(out=sg[:], in_=xs_all[64:65, :])
                    return {"it": it, "lqh": lqh, "dc": dc, "xs": xs, "sg": sg,
                            "last": last}

                def pv_mid(st):
                    nc.vector.reciprocal(st["sg"][:], st["sg"][:])
                    zrow = sgp.tile([1, 2048], bf16, tag="zrow")
                    nc.sync.dma_start(out=zrow[:], in_=st["sg"][:])
                    bc = {}
                    for j in range(4):
                        t = bcp.tile([64, 512], bf16, tag="bcp", name=f"bc{j}")
                        nc.gpsimd.partition_broadcast(
                            t[:], zrow[0:1, 512 * j : 512 * j + 512], channels=64
                        )
                        bc[j] = t
                    st["bc"] = bc

                def pv_finish(st):
                    lqh, dc, xs, bc = st["lqh"], st["dc"], st["xs"], st["bc"]
                    last = st["last"]
                    for hs in range(2):
                        jv, jr = 2 * hs, 2 * hs + 1
                        t1 = xsb.tile([65, 512], bf16, tag="xsb")
                        eng1 = nc.gpsimd if last else nc.vector
                        eng1.tensor_tensor(
                            out=t1[0:64, :], in0=xs[(0, hs)][0:64, :],
                            in1=bc[jv][:], op=Mult,
                        )
                        t2 = xsb.tile([65, 512], bf16, tag="xsb")
                        nc.vector.tensor_tensor(
                            out=t2[0:64, :], in0=xs[(1, hs)][0:64, :],
                            in1=bc[jr][:], op=Mult,
                        )
                        xf_slice = slice(
                            1024 * dc + 512 * lqh, 1024 * dc + 512 * lqh + 512
                        )
                        if hs == 0:
                            nc.vector.tensor_tensor(
                                out=xf_sb[0:64, xf_slice], in0=t1[0:64, :],
                                in1=t2[0:64, :], op=Add,
                            )
                        else:
                            t3 = xsb.tile([65, 512], bf16, tag="xsb")
                            eng3 = nc.gpsimd if last else nc.vector
                            eng3.tensor_tensor(
                                out=t3[0:64, :], in0=t1[0:64, :], in1=t2[0:64, :],
                                op=Add,
                            )
                            nc.sync.dma_start(
                                out=xf_sb[64:128, xf_slice], in_=t3[0:64, :]
                            )

                def y_evict(ps, ot, lqh, qno):
                    # PSUM->SBUF eviction on the ACT engine (idle by now), DMA
                    # alternating between the two HWDGE queues.
                    y = ysb.tile([128, 512], bf16, tag="ysb")
                    nc.scalar.activation(y[:], ps[:], Copy)
                    eng = nc.sync if qno % 2 == 0 else nc.scalar
                    eng.dma_start(
                        out=yT[128 * ot : 128 * ot + 128, 512 * lqh : 512 * lqh + 512],
                        in_=y[:],
                    )

                def emit_outproj0():
                    for ot in range(8):
                        ps = xpool.tile([128, 512], f32, tag="xpool", name=f"psy{ot}")
                        for dc in range(4):
                            nc.tensor.matmul(
                                ps[:],
                                wo_sb[:, 1024 * dc + 128 * ot : 1024 * dc + 128 * ot + 128],
                                xf_sb[:, 1024 * dc : 1024 * dc + 512],
                                start=(dc == 0),
                                stop=(dc == 3),
                            )
                        y_evict(ps, ot, 0, ot)

                def emit_outproj1():
                    # dc-outer with all 8 accumulators live (4 xpool banks + 4
                    # spool halves). dc=3 is further split into K=64 halves so
                    # only the hs=1 half waits for the final xf shift-DMA.
                    pss = []
                    for i in range(4):
                        pss.append(
                            xpool.tile([128, 512], f32, tag="xpool", name=f"psw{i}")
                        )
                    w3a = spool.tile([128, 1024], f32, tag="spool", name="psw3a")
                    w3b = spool.tile([128, 1024], f32, tag="spool", name="psw3b")
                    for w in (w3a, w3b):
                        pss.append(w[:, 0:512])
                        pss.append(w[:, 512:1024])
                    for dc in range(3):
                        for ot in range(8):
                            nc.tensor.matmul(
                                pss[ot][0:128, :],
                                wo_sb[:, 1024 * dc + 128 * ot : 1024 * dc + 128 * ot + 128],
                                xf_sb[:, 1024 * dc + 512 : 1024 * dc + 1024],
                                start=(dc == 0),
                                stop=False,
                            )
                    for hs in range(2):
                        psl = slice(64 * hs, 64 * hs + 64)
                        for ot in range(8):
                            nc.tensor.matmul(
                                pss[ot][0:128, :],
                                wo_sb[psl, 3072 + 128 * ot : 3072 + 128 * ot + 128],
                                xf_sb[psl, 3584:4096],
                                start=False,
                                stop=(hs == 1),
                            )
                    for ot in range(8):
                        y_evict(pss[ot], ot, 1, ot + 1)

                prev = None
                for lqh in range(2):
                    for dc in range(4):
                        last = lqh == 1 and dc == 3
                        st = pv_front(lqh, dc, use_spool=last, last=last)
                        if prev is not None:
                            pv_finish(prev)
                        pv_mid(st)
                        prev = st
                # outproj(0) has no dependency on the last normalize chain, so
                # the PE stays busy while that chain drains.
                emit_outproj0()
                pv_finish(prev)
                emit_outproj1()

    nc.compile()
    return nc


def _get_program(lkp=LKP):
    if lkp not in _CACHE:
        _CACHE[lkp] = _build_program(lkp)
    return _CACHE[lkp]


def _bf16(arr):
    import ml_dtypes

    return np.ascontiguousarray(np.asarray(arr, dtype=np.float32).astype(ml_dtypes.bfloat16))


def _pack_chunks(mat_t, nk, L):
    """[nk*128, L] -> [128, nk*L] with chunk k at cols [k*L, (k+1)*L)."""
    return mat_t.reshape(nk, 128, L).transpose(1, 0, 2).reshape(128, nk * L)


def _shard_inputs(inputs, lkp=LKP):
    q = np.ascontiguousarray(inputs["query"], dtype=np.float32)
    k = np.ascontiguousarray(inputs["key"], dtype=np.float32)
    v = np.ascontiguousarray(inputs["value"], dtype=np.float32)
    wr = np.ascontiguousarray(inputs["weak_rela"], dtype=np.float32)
    mask = np.asarray(inputs["mask"])

    def t_weight(W, hsl):
        # Wt [D, HD] -> [128, 4096] with col = dc*1024 + k*128 + c
        Wt = np.asarray(W, dtype=np.float32)[hsl, :].T
        return Wt.reshape(8, 128, 4, 128).transpose(1, 2, 0, 3).reshape(128, 4096)

    def n_weight(W, hsl):
        # Wt [D, HD] -> [128, 4096] with col = k*512 + c
        Wt = np.asarray(W, dtype=np.float32)[hsl, :].T
        return _pack_chunks(Wt, 8, 512)

    in_maps = []
    for c in range(N_CORES):
        b, hh = divmod(c, 2)
        hsl = slice(HD * hh, HD * hh + HD)
        idx = np.nonzero(mask[b, 0])[0]
        nv = len(idx)
        assert nv <= lkp
        pidx = np.concatenate([idx, np.zeros(lkp - nv, dtype=idx.dtype)])
        bias = np.full(lkp, -1.0e9, np.float32)
        bias[:nv] = 0.0
        mb = np.ascontiguousarray(bias.reshape(lkp // 128, 128).T)
        kc, vc, wrc = k[b][pidx], v[b][pidx], wr[b][pidx]
        # wo: [HD, D] -> [128, 4096] with col = dc*1024 + c
        woT = np.asarray(inputs["Wo"], dtype=np.float32)[:, hsl].T
        wo_p = _pack_chunks(woT, 4, 1024)
        m = {
            "xq": _bf16(_pack_chunks(q[b].T, 8, LQ)),
            "xk": _bf16(_pack_chunks(kc.T, 8, lkp)),
            "xr": _bf16(_pack_chunks(wrc.T, 8, lkp)),
            "xv": _bf16(_pack_chunks(vc.T, 8, lkp)),
            "wq": _bf16(t_weight(inputs["Wq"], hsl)),
            "wk": _bf16(t_weight(inputs["Wk"], hsl)),
            "wrk": _bf16(t_weight(inputs["Wrk"], hsl)),
            "wv": _bf16(n_weight(inputs["Wv"], hsl)),
            "wrv": _bf16(n_weight(inputs["Wrv"], hsl)),
            "wo": _bf16(wo_p),
            "bq_pc": np.asarray(inputs["bq"][hsl]).reshape(4, 128).T.astype(np.float32),
            "bk_pc": np.asarray(inputs["bk"][hsl]).reshape(4, 128).T.astype(np.float32),
            "brk_pc": np.asarray(inputs["brk"][hsl]).reshape(4, 128).T.astype(np.float32),
            "bv_bc": np.broadcast_to(inputs["bv"][hsl], (128, HD)).astype(np.float32),
            "brv_bc": np.broadcast_to(inputs["brv"][hsl], (128, HD)).astype(np.float32),
            "maskb": mb,
        }
        in_maps.append({k2: np.ascontiguousarray(v2) for k2, v2 in m.items()})
    return in_maps


def run_on_hw(inputs, trace=False, **kw):
    from concourse.bass_utils import run_bass_kernel_spmd

    mask = np.asarray(inputs["mask"])
    max_valid = max(int(mask[b, 0].sum()) for b in range(B))
    lkp = max(LKP, ((max_valid + 127) // 128) * 128)
    nc = _get_program(lkp)
    in_maps = _shard_inputs(inputs, lkp)
    res = run_bass_kernel_spmd(
        nc, in_maps, core_ids=list(range(N_CORES)), trace=trace, **kw
    )
    bo = np.asarray(inputs["bo"], dtype=np.float32)
    outs = []
    for b in range(B):
        yt = res.results[2 * b]["yT"].astype(np.float32) + res.results[
            2 * b + 1
        ]["yT"].astype(np.float32)
        outs.append(yt.T + bo)
    out = np.stack(outs).astype(np.float32)
    return out, res


def kernel(**inputs):
    out, _ = run_on_hw(inputs)
    return out
